# revision 1
# baseline (speedup 1.0000x reference)
"""Trainium2 Bass kernel for NemotronH native MoE (T=2048, H=2048, E=32,
DF=1024, DS=4096, top-k=6, sigmoid router with group-limited routing).

Strategy (8 NeuronCores, full I/O):
  - Router + top-k run on host in fp32 numpy (bit-identical expert selection
    to the jax reference; verified).
  - Expert parallelism: 32 routed experts bin-packed 4-per-core into 4
    "slots"; host gathers each expert's tokens into a transposed, padded
    activation block.  Slot capacities come from the actual routing and are
    baked into the Bass program (built per call, cached by capacity tuple).
  - Routed experts are software-pipelined: up[j+1] is emitted before
    down[j] so the PE never waits for PSUM bank turnover at boundaries.
  - Shared expert: 4-way tensor-parallel over DS x 2-way data-parallel over
    tokens (core c: token half c//4, DS quarter c%4); partials summed on host.
  - Matmuls in bf16 (full-rate PE, FWL weight loads), fp32 PSUM accumulate,
    fp32 outputs.  DMA spread over three HWDGE queues (sync=weights,
    scalar=activations, vector=outputs) to avoid head-of-line stalls.
"""

import os
import sys
import numpy as np

try:
    import concourse.bacc as bacc  # noqa: F401
except ImportError:
    sys.path.insert(0, "/opt/trn_rl_repo")

import concourse.bacc as bacc
import concourse.tile as tile
from concourse import mybir
from concourse.bass_utils import run_bass_kernel_spmd

# ---- problem constants (hardcoded per contest rules) ----
T = 2048
H = 2048
E = 32
DF = 1024
DS = 4096
TOP_K = 6
N_GROUP = 8
TOPK_GROUP = 4
SCALE = 2.5
N_CORES = 8
SLOTS = 4         # routed experts per core
TP_S = 4          # shared expert: tensor-parallel degree over DS
DP_S = N_CORES // TP_S   # shared expert: token-parallel degree
DS_LOC = DS // TP_S      # 1024
T_LOC = T // DP_S        # 1024

UP_DT = mybir.dt.bfloat16    # wu, xt, su, xts
DOWN_DT = mybir.dt.bfloat16  # wd, sd, relu2 activations
F32 = mybir.dt.float32

LAST_RESULTS = None
LAST_EXEC_NS = None

_OUT_ENG = os.environ.get("K_OUT_ENGINE", "sync")   # sync | scalar | gpsimd
_XT_ENG = os.environ.get("K_XT_ENGINE", "scalar")     # scalar | sync
_PIPE = os.environ.get("K_PIPE", "1") == "1"

_PROG_CACHE = {}


def _route_host(x, router_w, router_b):
    """fp32 numpy replica of reference._route (verified bit-identical tidx)."""
    logits = x @ router_w.T
    scores = (1.0 / (1.0 + np.exp(-logits))).astype(np.float32)
    sfc = scores + router_b[None, :]
    gsize = E // N_GROUP
    grp = sfc.reshape(T, N_GROUP, gsize)
    g2 = -np.sort(-grp, axis=-1)[:, :, :2]
    group_scores = g2.sum(-1)
    gidx = np.argsort(-group_scores, axis=-1, kind="stable")[:, :TOPK_GROUP]
    group_mask = np.zeros((T, N_GROUP), dtype=sfc.dtype)
    np.put_along_axis(group_mask, gidx, 1.0, axis=1)
    score_mask = np.repeat(group_mask, gsize, axis=1)
    masked = np.where(score_mask > 0, sfc, 0.0)
    tidx = np.argsort(-masked, axis=-1, kind="stable")[:, :TOP_K].astype(np.int32)
    tw = np.take_along_axis(scores, tidx, axis=1)
    tw = tw / (tw.sum(-1, keepdims=True) + 1e-20)
    tw = (tw * SCALE).astype(np.float32)
    return tidx, tw


def _roundup(v, m):
    return -(-v // m) * m


def _up_chunks(c):
    """Split token count c into <=512-wide pieces for the up-GEMM rhs."""
    n = max(1, -(-c // 512))
    base, rem = divmod(c, n)
    widths = [base + (1 if i < rem else 0) for i in range(n)]
    out, off = [], 0
    for w in widths:
        out.append((off, w))
        off += w
    return out


def _build_program(caps):
    nc = bacc.Bacc("TRN2", target_bir_lowering=False, debug=False,
                   num_devices=N_CORES)

    xt_r = [nc.dram_tensor(f"xt{j}", [H, caps[j]], UP_DT, kind="ExternalInput")
            for j in range(SLOTS)]
    cw_r = [nc.dram_tensor(f"cw{j}", [caps[j], 1], F32, kind="ExternalInput")
            for j in range(SLOTS)]
    wu = nc.dram_tensor("wu", [SLOTS, H, DF], UP_DT, kind="ExternalInput")
    wd = nc.dram_tensor("wd", [SLOTS, DF, H], DOWN_DT, kind="ExternalInput")
    su = nc.dram_tensor("su", [H, DS_LOC], UP_DT, kind="ExternalInput")
    sd = nc.dram_tensor("sd", [DS_LOC, H], DOWN_DT, kind="ExternalInput")
    xts = nc.dram_tensor("xts", [H, T_LOC], UP_DT, kind="ExternalInput")
    yr = [nc.dram_tensor(f"yr{j}", [caps[j], H], F32, kind="ExternalOutput")
          for j in range(SLOTS)]
    ys = nc.dram_tensor("ys", [T_LOC, H], F32, kind="ExternalOutput")

    KH = H // 128      # 16 k-tiles over H
    KD = DF // 128     # 8 k-tiles over DF (down contraction)
    MD = DF // 128     # 8 m-tiles over DF
    NH = H // 512      # 4 n-chunks over H
    relu = mybir.ActivationFunctionType.Relu

    with tile.TileContext(nc) as tc:
        MS = DS_LOC // 128   # 8 m-tiles over DS_LOC
        NT = T_LOC // 512    # 2 token chunks (shared up rhs)
        KS = DS_LOC // 128   # 8 k-tiles (shared down contraction)
        with (
            tc.tile_pool(name="ps", bufs=8, space="PSUM") as pp,
            tc.tile_pool(name="as_", bufs=MS) as asp,
            tc.tile_pool(name="sd", bufs=KS) as sdp,
        ):
            # ---- shared expert up (TP over DS x DP over T): runs first ----
            a_s = [asp.tile([128, T_LOC], DOWN_DT, tag="as", name=f"as{m}")
                   for m in range(MS)]
            with (
                tc.tile_pool(name="su", bufs=KH) as sup,
                tc.tile_pool(name="xn", bufs=KH) as xnp,
                tc.tile_pool(name="rs", bufs=6) as rsp,
            ):
                su_tiles = []
                xn_tiles = []
                for k in range(KH):
                    t = sup.tile([128, DS_LOC], UP_DT, tag="su", name=f"su{k}")
                    nc.sync.dma_start(t[:], su.ap()[k * 128:(k + 1) * 128, :])
                    su_tiles.append(t)
                    t2 = xnp.tile([128, T_LOC], UP_DT, tag="xn", name=f"xn{k}")
                    nc.sync.dma_start(t2[:],
                                      xts.ap()[k * 128:(k + 1) * 128, :])
                    xn_tiles.append(t2)
                for m in range(MS):
                    for n in range(NT):
                        ps = pp.tile([128, 512], F32, tag="ps",
                                     name=f"psh{m}_{n}")
                        for k in range(KH):
                            nc.tensor.matmul(
                                ps[:],
                                su_tiles[k][:, m * 128:(m + 1) * 128],
                                xn_tiles[k][:, n * 512:(n + 1) * 512],
                                start=(k == 0), stop=(k == KH - 1))
                        r = rsp.tile([128, 512], DOWN_DT, tag="rs",
                                     name=f"rs{m}_{n}")
                        nc.scalar.activation(r[:], ps[:], relu)
                        nc.vector.tensor_mul(
                            a_s[m][:, n * 512:(n + 1) * 512], r[:], r[:])
            # shared-down weights: prefetch during the routed section
            sd_tiles = []
            for k2 in range(KS):
                t = sdp.tile([128, H], DOWN_DT, tag="sd", name=f"sd{k2}")
                nc.sync.dma_start(t[:], sd.ap()[k2 * 128:(k2 + 1) * 128, :])
                sd_tiles.append(t)

            # ---------------- routed experts (pipelined) ----------------
            with (
                tc.tile_pool(name="wu", bufs=4) as wup,
                tc.tile_pool(name="wd", bufs=9) as wdp,
                tc.tile_pool(name="xt", bufs=34) as xtp,
                tc.tile_pool(name="at", bufs=20) as atp,
                tc.tile_pool(name="rl", bufs=6) as rlp,
                tc.tile_pool(name="cw", bufs=8) as cwp,
                tc.tile_pool(name="os", bufs=3) as osp,
            ):
                state = {}
                xt_loaded = {}

                def load_xt(j):
                    if j in xt_loaded or j >= SLOTS:
                        return
                    C = caps[j]
                    tiles = []
                    for k in range(KH):
                        t = xtp.tile([128, C], UP_DT, tag="xt",
                                     name=f"xt{j}_{k}")
                        nc.sync.dma_start(
                            t[:], xt_r[j].ap()[k * 128:(k + 1) * 128, :])
                        tiles.append(t)
                    xt_loaded[j] = tiles

                def emit_up(j):
                    C = caps[j]
                    load_xt(j)
                    xt_tiles = xt_loaded[j]
                    a_tiles = [atp.tile([128, C], DOWN_DT, tag="at",
                                        name=f"a{j}_{m}") for m in range(MD)]
                    first = True
                    for (off, w) in _up_chunks(C):
                        psums = [pp.tile([128, w], F32, tag="ps",
                                         name=f"ph{j}_{m}") for m in range(MD)]
                        for k in range(KH):
                            wu_t = wup.tile([128, DF], UP_DT, tag="wu",
                                            name=f"wu{j}_{k}")
                            nc.sync.dma_start(
                                wu_t[:], wu.ap()[j, k * 128:(k + 1) * 128, :])
                            for m in range(MD):
                                nc.tensor.matmul(
                                    psums[m][:],
                                    wu_t[:, m * 128:(m + 1) * 128],
                                    xt_tiles[k][:, off:off + w],
                                    start=(k == 0), stop=(k == KH - 1))
                        if first:
                            # prefetch next expert's tokens behind chunk 0
                            load_xt(j + 1)
                            first = False
                        for m in range(MD):
                            r = rlp.tile([128, w], DOWN_DT, tag="rl",
                                         name=f"r{j}_{m}")
                            nc.scalar.activation(r[:], psums[m][:], relu)
                            nc.vector.tensor_mul(
                                a_tiles[m][:, off:off + w], r[:], r[:])
                    # prefetch this expert's w_down right after its up block
                    wd_tiles = []
                    for k2 in range(KD):
                        t = wdp.tile([128, H], DOWN_DT, tag="wd",
                                     name=f"wd{j}_{k2}")
                        nc.sync.dma_start(
                            t[:], wd.ap()[j, k2 * 128:(k2 + 1) * 128, :])
                        wd_tiles.append(t)
                    state[j] = (a_tiles, wd_tiles)
                    del xt_loaded[j]

                def emit_down(j):
                    C = caps[j]
                    a_tiles, wd_tiles = state.pop(j)
                    n_tc = -(-C // 128)
                    for tci in range(n_tc):
                        t0 = tci * 128
                        M = min(128, C - t0)
                        cw_t = cwp.tile([128, 1], F32, tag="cw",
                                        name=f"cw{j}_{tci}")
                        nc.sync.dma_start(cw_t[:M, :],
                                          cw_r[j].ap()[t0:t0 + M, :])
                        os_t = osp.tile([128, H], F32, tag="os",
                                        name=f"os{j}_{tci}")
                        for nn in range(NH):
                            ps = pp.tile([128, 512], F32, tag="ps",
                                         name=f"pd{j}_{tci}_{nn}")
                            for k2 in range(KD):
                                nc.tensor.matmul(
                                    ps[:M, :],
                                    a_tiles[k2][:, t0:t0 + M],
                                    wd_tiles[k2][:, nn * 512:(nn + 1) * 512],
                                    start=(k2 == 0), stop=(k2 == KD - 1))
                            nc.vector.tensor_scalar_mul(
                                os_t[:M, nn * 512:(nn + 1) * 512], ps[:M, :],
                                cw_t[:M, :])
                        getattr(nc, _OUT_ENG).dma_start(
                            yr[j].ap()[t0:t0 + M, :], os_t[:M, :])

                if _PIPE:
                    emit_up(0)
                    emit_up(1)
                    emit_down(0)
                    emit_up(2)
                    emit_down(1)
                    emit_up(3)
                    emit_down(2)
                    emit_down(3)
                else:
                    for j in range(SLOTS):
                        emit_up(j)
                        emit_down(j)

            # ---------------- shared expert down: runs last ----------------
            with tc.tile_pool(name="ss", bufs=3) as ssp:
                for tci in range(T_LOC // 128):
                    t0 = tci * 128
                    ss_t = ssp.tile([128, H], F32, tag="ss", name=f"ss{tci}")
                    for nn in range(NH):
                        ps = pp.tile([128, 512], F32, tag="ps",
                                     name=f"psd{tci}_{nn}")
                        for k2 in range(KS):
                            nc.tensor.matmul(
                                ps[:],
                                a_s[k2][:, t0:t0 + 128],
                                sd_tiles[k2][:, nn * 512:(nn + 1) * 512],
                                start=(k2 == 0), stop=(k2 == KS - 1))
                        nc.vector.tensor_copy(
                            ss_t[:, nn * 512:(nn + 1) * 512], ps[:])
                    getattr(nc, _OUT_ENG).dma_start(
                        ys.ap()[t0:t0 + 128, :], ss_t[:])

    nc.compile()
    return nc


def kernel(x, router_w, router_b, w_up, w_down, shared_up, shared_down):
    global LAST_RESULTS, LAST_EXEC_NS
    x = np.asarray(x, dtype=np.float32)
    router_w = np.asarray(router_w, dtype=np.float32)
    router_b = np.asarray(router_b, dtype=np.float32)
    w_up = np.asarray(w_up, dtype=np.float32)
    w_down = np.asarray(w_down, dtype=np.float32)
    shared_up = np.asarray(shared_up, dtype=np.float32)
    shared_down = np.asarray(shared_down, dtype=np.float32)

    tidx, tw = _route_host(x, router_w, router_b)

    tok_of = [None] * E
    wgt_of = [None] * E
    for e in range(E):
        rows, cols = np.nonzero(tidx == e)
        tok_of[e] = rows
        wgt_of[e] = tw[rows, cols]
    counts = np.array([len(tok_of[e]) for e in range(E)])

    # bin-pack: rank groups of 8 per slot; greedy core assignment for balance
    order = np.argsort(-counts, kind="stable")
    assign = np.zeros((N_CORES, SLOTS), dtype=np.int64)
    core_load = np.zeros(N_CORES, dtype=np.int64)
    caps = []
    for j in range(SLOTS):
        grp = order[j * N_CORES:(j + 1) * N_CORES]
        caps.append(int(_roundup(max(int(counts[grp].max()), 16), 8)))
        cores_by_load = np.argsort(core_load, kind="stable")
        for i, e in enumerate(grp):  # grp is desc; pair big with least-loaded
            c = cores_by_load[i]
            assign[c, j] = e
            core_load[c] += counts[e]
    caps = tuple(caps)

    np_up = np.float32 if UP_DT in (mybir.dt.float32, mybir.dt.float32r) \
        else mybir.dt.np(UP_DT)
    np_dn = np.float32 if DOWN_DT in (mybir.dt.float32, mybir.dt.float32r) \
        else mybir.dt.np(DOWN_DT)

    xt_full = np.ascontiguousarray(x.T)
    xt_full_cast = xt_full.astype(np_up, copy=False)
    wu_cast = w_up.astype(np_up, copy=False)
    wd_cast = w_down.astype(np_dn, copy=False)
    su_cast = shared_up.astype(np_up, copy=False)
    sd_cast = shared_down.astype(np_dn, copy=False)

    in_maps = []
    for c in range(N_CORES):
        m = {}
        exp_ids = assign[c]
        for j in range(SLOTS):
            e = exp_ids[j]
            n = counts[e]
            xt_cj = np.zeros((H, caps[j]), dtype=np_up)
            xt_cj[:, :n] = xt_full_cast[:, tok_of[e]]
            cw_cj = np.zeros((caps[j], 1), dtype=np.float32)
            cw_cj[:n, 0] = wgt_of[e]
            m[f"xt{j}"] = xt_cj
            m[f"cw{j}"] = cw_cj
        m["wu"] = np.ascontiguousarray(wu_cast[exp_ids])
        m["wd"] = np.ascontiguousarray(wd_cast[exp_ids])
        r_tp = c % TP_S
        g_dp = c // TP_S
        m["su"] = np.ascontiguousarray(
            su_cast[:, r_tp * DS_LOC:(r_tp + 1) * DS_LOC])
        m["sd"] = np.ascontiguousarray(
            sd_cast[r_tp * DS_LOC:(r_tp + 1) * DS_LOC, :])
        m["xts"] = np.ascontiguousarray(
            xt_full_cast[:, g_dp * T_LOC:(g_dp + 1) * T_LOC])
        in_maps.append(m)

    key = (caps, str(UP_DT), str(DOWN_DT))
    nc = _PROG_CACHE.get(key)
    if nc is None:
        nc = _build_program(caps)
        _PROG_CACHE[key] = nc

    res = run_bass_kernel_spmd(nc, in_maps, list(range(N_CORES)))
    LAST_RESULTS = res
    LAST_EXEC_NS = res.exec_time_ns

    out = np.zeros((T, H), dtype=np.float64)
    for c in range(N_CORES):
        g_dp = c // TP_S
        out[g_dp * T_LOC:(g_dp + 1) * T_LOC] += \
            res.results[c]["ys"].astype(np.float64)
        for j in range(SLOTS):
            e = assign[c, j]
            n = counts[e]
            if n:
                # token rows are unique within one expert's list
                out[tok_of[e]] += res.results[c][f"yr{j}"][:n].astype(np.float64)
    return out.astype(np.float32)



# revision 7
# speedup vs baseline: 1.2685x; 1.2685x over previous
"""Trainium2 Bass kernel for NemotronH native MoE (T=2048, H=2048, E=32,
DF=1024, DS=4096, top-k=6, sigmoid router with group-limited routing).

Strategy (8 NeuronCores, full I/O):
  - Router + top-k run on host in fp32 numpy (bit-identical expert selection
    to the jax reference).
  - Expert parallelism: 32 routed experts bin-packed 4-per-core into 4
    "slots"; host gathers each expert's tokens into a transposed, padded
    activation block.  Slot capacities are baked into the Bass program
    (built per call, cached by capacity tuple).
  - All device inputs/outputs are host-prepacked into partition-major
    [128, X] layouts so each tensor moves with O(1) large DMAs instead of
    per-k-tile descriptors (DMA issue is ~0.7us/instruction on a queue).
  - Weight DMAs ride the gpsimd queue, activations the sync queue, outputs
    the scalar queue (same queue as the PSUM->SBUF copies that produce
    them, so no cross-queue head-of-line blocking).
  - GEMM loops are ordered so one LDWEIGHTS feeds multiple 512-wide
    matmuls (down/shared: k-outer, n-inner with 4 live PSUM banks), and
    consumption order follows the DMA arrival wave.
  - Down-projections consume a-tiles in production order (k2 == m), so
    up(j) -> down(j) chains with no PE bubble and the HAM clock gate never
    re-throttles mid-kernel.
  - Combine weights are folded into the PSUM->SBUF copy on the scalar
    engine (activation Copy with per-partition scale).
  - Shared expert: 4-way tensor-parallel over DS x 2-way data-parallel
    over tokens; partials summed on host.
  - Matmuls in bf16 (full-rate PE + FWL), fp32 PSUM accumulate, fp32 out.
"""

import sys
import numpy as np

try:
    import concourse.bacc as bacc  # noqa: F401
except ImportError:
    sys.path.insert(0, "/opt/trn_rl_repo")

import concourse.bacc as bacc
import concourse.tile as tile
from concourse import mybir
from concourse.bass_utils import run_bass_kernel_spmd

# ---- problem constants (hardcoded per contest rules) ----
T = 2048
H = 2048
E = 32
DF = 1024
DS = 4096
TOP_K = 6
N_GROUP = 8
TOPK_GROUP = 4
SCALE = 2.5
N_CORES = 8
SLOTS = 4          # routed experts per core
TP_S = 4           # shared expert: tensor-parallel degree over DS
DP_S = N_CORES // TP_S   # shared expert: token-parallel degree
DS_LOC = DS // TP_S      # 1024
T_LOC = T // DP_S        # 1024

KH = H // 128      # 16 k-tiles over H
MD = DF // 128     # 8 m-tiles over DF
KD = DF // 128     # 8 k-tiles over DF (down contraction)
NH = H // 512      # 4 n-chunks over H
MS = DS_LOC // 128  # 8 m-tiles over DS_LOC
KS = DS_LOC // 128  # 8 k-tiles over DS_LOC (shared down contraction)
NT = T_LOC // 512  # 2 token chunks (shared up rhs)

BF16 = mybir.dt.bfloat16
F32 = mybir.dt.float32

LAST_RESULTS = None
LAST_EXEC_NS = None

_PROG_CACHE = {}


def _route_host(x, router_w, router_b):
    """fp32 numpy replica of reference._route (bit-identical tidx)."""
    logits = x @ router_w.T
    scores = (1.0 / (1.0 + np.exp(-logits))).astype(np.float32)
    sfc = scores + router_b[None, :]
    gsize = E // N_GROUP
    grp = sfc.reshape(T, N_GROUP, gsize)
    g2 = -np.sort(-grp, axis=-1)[:, :, :2]
    group_scores = g2.sum(-1)
    gidx = np.argsort(-group_scores, axis=-1, kind="stable")[:, :TOPK_GROUP]
    group_mask = np.zeros((T, N_GROUP), dtype=sfc.dtype)
    np.put_along_axis(group_mask, gidx, 1.0, axis=1)
    score_mask = np.repeat(group_mask, gsize, axis=1)
    masked = np.where(score_mask > 0, sfc, 0.0)
    tidx = np.argsort(-masked, axis=-1, kind="stable")[:, :TOP_K].astype(np.int32)
    tw = np.take_along_axis(scores, tidx, axis=1)
    tw = tw / (tw.sum(-1, keepdims=True) + 1e-20)
    tw = (tw * SCALE).astype(np.float32)
    return tidx, tw


def _roundup(v, m):
    return -(-v // m) * m


def _up_chunks(cap):
    """Token chunks (<=512 wide) for the up-GEMM moving operand."""
    if cap <= 512:
        return [(0, cap)]
    assert cap <= 1024
    return [(0, 512), (512, cap - 512)]


def _up_layout(cap):
    """(chunks, m_groups, block order) for the up pass of one slot.

    One PSUM bank per (m, chunk); the live set per m_group must be <= 8,
    so 2-chunk slots process DF m-tiles in halves.  The returned block
    order is the wu 128-col-block consumption order (shared with the host
    packer).
    """
    chunks = _up_chunks(cap)
    if len(chunks) == 1:
        m_groups = [list(range(MD))]
    else:
        m_groups = [list(range(0, MD // 2)), list(range(MD // 2, MD))]
    order = [(k, m) for mg in m_groups for k in range(KH) for m in mg]
    return chunks, m_groups, order


def _build_program(caps):
    ntiles = [-(-c // 128) for c in caps]
    nc = bacc.Bacc("TRN2", target_bir_lowering=False, debug=False,
                   num_devices=N_CORES)

    xt_r = [nc.dram_tensor(f"xt{j}", [128, KH * caps[j]], BF16,
                           kind="ExternalInput") for j in range(SLOTS)]
    cw_r = [nc.dram_tensor(f"cw{j}", [128, ntiles[j]], F32,
                           kind="ExternalInput") for j in range(SLOTS)]
    wu = nc.dram_tensor("wu", [SLOTS, 128, KH * DF], BF16,
                        kind="ExternalInput")
    wd = nc.dram_tensor("wd", [SLOTS, 128, KD * H], BF16,
                        kind="ExternalInput")
    su = nc.dram_tensor("su", [128, MS * KH * 128], BF16,
                        kind="ExternalInput")
    sd = nc.dram_tensor("sd", [128, KS * H], BF16, kind="ExternalInput")
    xts = nc.dram_tensor("xts", [128, KH * T_LOC], BF16,
                         kind="ExternalInput")
    yr = [nc.dram_tensor(f"yr{j}", [128, ntiles[j] * H], F32,
                         kind="ExternalOutput") for j in range(SLOTS)]
    ys = nc.dram_tensor("ys", [128, (T_LOC // 128) * H], F32,
                        kind="ExternalOutput")

    relu = mybir.ActivationFunctionType.Relu
    copyf = mybir.ActivationFunctionType.Copy
    CAPMAX = max(caps)

    with tile.TileContext(nc) as tc:
        with (
            tc.tile_pool(name="pp", bufs=8, space="PSUM") as pp,
            tc.tile_pool(name="xt", bufs=2) as xtp,        # [128,16*cap] bf16
            tc.tile_pool(name="wu", bufs=3) as wup,        # quarters, 8KB
            tc.tile_pool(name="wd", bufs=2) as wdp,        # halves, 8KB
            tc.tile_pool(name="su", bufs=6) as sup,        # m-blocks, 4KB
            tc.tile_pool(name="sx", bufs=1) as sxp,        # xts / sd, 32KB
            tc.tile_pool(name="as_", bufs=1) as asp,       # a_s, 16KB
            tc.tile_pool(name="at", bufs=2) as atp,        # a-tiles, 8.6KB
            tc.tile_pool(name="os", bufs=2) as osp,        # out stage, 8KB
            tc.tile_pool(name="rl", bufs=4) as rlp,        # relu tmp, 1KB
            tc.tile_pool(name="cw", bufs=4) as cwp,
        ):
            # ---------- activation / small input DMAs (sync queue) ----------
            xt_t = {}
            for j in range(2):
                xt_t[j] = xtp.tile([128, KH * caps[j]], BF16, tag="xt",
                                   name=f"xt{j}")
                nc.sync.dma_start(xt_t[j][:], xt_r[j].ap()[:, :])
            xts_t = sxp.tile([128, KH * T_LOC], BF16, tag="sx", name="xts")
            nc.sync.dma_start(xts_t[:], xts.ap()[:, :])
            cw_t = []
            for j in range(SLOTS):
                t = cwp.tile([128, ntiles[j]], F32, tag="cw", name=f"cw{j}")
                nc.sync.dma_start(t[:], cw_r[j].ap()[:, :])
                cw_t.append(t)

            # ---------- weight DMA emitters (gpsimd queue) ----------
            NQ = 4                       # wu quarters per expert
            QW = KH * DF // NQ           # 4096 cols per quarter
            NWH = 2                      # wd halves per expert
            HW_ = KD * H // NWH          # 8192 cols per half

            def load_wu(j):
                qs = []
                for q in range(NQ):
                    t = wup.tile([128, QW], BF16, tag="wu", name=f"wu{j}_{q}")
                    nc.gpsimd.dma_start(
                        t[:], wu.ap()[j, :, q * QW:(q + 1) * QW])
                    qs.append(t)
                return qs

            def load_wd(j):
                hs = []
                for hh in range(NWH):
                    t = wdp.tile([128, HW_], BF16, tag="wd", name=f"wd{j}_{hh}")
                    nc.gpsimd.dma_start(
                        t[:], wd.ap()[j, :, hh * HW_:(hh + 1) * HW_])
                    hs.append(t)
                return hs

            # program-order state
            a_t = {}
            wd_t = {}

            def emit_up(j, wu_q):
                cap = caps[j]
                chunks, m_groups, order = _up_layout(cap)
                bidx = {km: i for i, km in enumerate(order)}
                at_tile = atp.tile([128, MD * CAPMAX], BF16, tag="at",
                                   name=f"at{j}")
                for mg in m_groups:
                    ps = {}
                    for m in mg:
                        for ci in range(len(chunks)):
                            ps[(m, ci)] = pp.tile([128, 512], F32, tag="pp",
                                                  name=f"pu{j}_{m}_{ci}")
                    for k in range(KH):
                        for m in mg:
                            bi = bidx[(k, m)]
                            q, r = divmod(bi * 128, QW)
                            wsl = wu_q[q][:, r:r + 128]
                            for ci, (off, w) in enumerate(chunks):
                                nc.tensor.matmul(
                                    ps[(m, ci)][:, :w], wsl,
                                    xt_t[j][:, k * cap + off:k * cap + off + w],
                                    start=(k == 0), stop=(k == KH - 1))
                    for m in mg:
                        for ci, (off, w) in enumerate(chunks):
                            r = rlp.tile([128, 512], BF16, tag="rl",
                                         name=f"r{j}_{m}_{ci}")
                            nc.scalar.activation(r[:, :w], ps[(m, ci)][:, :w],
                                                 relu)
                            nc.vector.tensor_mul(
                                at_tile[:, m * cap + off:m * cap + off + w],
                                r[:, :w], r[:, :w])
                a_t[j] = at_tile
                del xt_t[j]

            def emit_down(j):
                cap = caps[j]
                at_tile = a_t.pop(j)
                wdh = wd_t.pop(j)
                for tci in range(ntiles[j]):
                    t0 = tci * 128
                    M = min(128, cap - t0)
                    ps = [pp.tile([128, 512], F32, tag="pp",
                                  name=f"pd{j}_{tci}_{nn}") for nn in range(NH)]
                    for k2 in range(KD):
                        hh, r = divmod(k2 * H, HW_)
                        asl = at_tile[:, k2 * cap + t0:k2 * cap + t0 + M]
                        for nn in range(NH):
                            nc.tensor.matmul(
                                ps[nn][:M, :], asl,
                                wdh[hh][:, r + nn * 512:r + (nn + 1) * 512],
                                start=(k2 == 0), stop=(k2 == KD - 1))
                    os_t = osp.tile([128, H], F32, tag="os",
                                    name=f"os{j}_{tci}")
                    for nn in range(NH):
                        nc.scalar.activation(
                            os_t[:M, nn * 512:(nn + 1) * 512], ps[nn][:M, :],
                            copyf, scale=cw_t[j][:M, tci:tci + 1])
                    nc.scalar.dma_start(yr[j].ap()[:M, tci * H:(tci + 1) * H],
                                        os_t[:M, :])

            # ================= schedule =================
            # up(0) first (xt0+wu0 arrive fastest), shared-up second (covers
            # wd0 transfer), then down/up chain, shared-down last.
            wu_t = {0: load_wu(0)}
            su_t = [sup.tile([128, KH * 128], BF16, tag="su", name=f"su{m}")
                    for m in range(MS)]
            for m in range(MS):
                nc.gpsimd.dma_start(
                    su_t[m][:], su.ap()[:, m * (KH * 128):(m + 1) * (KH * 128)])

            emit_up(0, wu_t.pop(0))

            # prefetches during shared-up
            wd_t[0] = load_wd(0)
            wu_t[1] = load_wu(1)

            # ---------------- shared expert up ----------------
            a_s = asp.tile([128, MS * T_LOC], BF16, tag="as", name="as")
            for m in range(MS):
                ps = [pp.tile([128, 512], F32, tag="pp", name=f"psu{m}_{n}")
                      for n in range(NT)]
                for k in range(KH):
                    wsl = su_t[m][:, k * 128:(k + 1) * 128]
                    for n in range(NT):
                        nc.tensor.matmul(
                            ps[n][:], wsl,
                            xts_t[:, k * T_LOC + n * 512:k * T_LOC + (n + 1) * 512],
                            start=(k == 0), stop=(k == KH - 1))
                for n in range(NT):
                    r = rlp.tile([128, 512], BF16, tag="rl", name=f"rs{m}_{n}")
                    nc.scalar.activation(r[:], ps[n][:], relu)
                    nc.vector.tensor_mul(
                        a_s[:, m * T_LOC + n * 512:m * T_LOC + (n + 1) * 512],
                        r[:], r[:])

            # sd arrives while routed experts run (reuses the xts buffer)
            sd_t = sxp.tile([128, KS * H], BF16, tag="sx", name="sd")
            nc.gpsimd.dma_start(sd_t[:], sd.ap()[:, :])

            # ---------------- routed experts ----------------
            for j in range(SLOTS):
                if j + 2 < SLOTS:
                    xt_t[j + 2] = xtp.tile([128, KH * caps[j + 2]], BF16,
                                           tag="xt", name=f"xt{j + 2}")
                    nc.sync.dma_start(xt_t[j + 2][:], xt_r[j + 2].ap()[:, :])
                if j > 0:
                    emit_up(j, wu_t.pop(j))
                if j + 1 < SLOTS:
                    wd_t[j + 1] = load_wd(j + 1)
                    if j + 2 < SLOTS:
                        wu_t[j + 2] = load_wu(j + 2)
                emit_down(j)

            # ---------------- shared expert down ----------------
            for tci in range(T_LOC // 128):
                t0 = tci * 128
                ps = [pp.tile([128, 512], F32, tag="pp", name=f"psd{tci}_{nn}")
                      for nn in range(NH)]
                for k2 in range(KS):
                    asl = a_s[:, k2 * T_LOC + t0:k2 * T_LOC + t0 + 128]
                    for nn in range(NH):
                        nc.tensor.matmul(
                            ps[nn][:], asl,
                            sd_t[:, k2 * H + nn * 512:k2 * H + (nn + 1) * 512],
                            start=(k2 == 0), stop=(k2 == KS - 1))
                os_t = osp.tile([128, H], F32, tag="os", name=f"oss{tci}")
                for nn in range(NH):
                    nc.scalar.activation(os_t[:, nn * 512:(nn + 1) * 512],
                                         ps[nn][:], copyf)
                nc.scalar.dma_start(ys.ap()[:, tci * H:(tci + 1) * H], os_t[:])

    nc.compile()
    return nc


def _pack_pm(mat, kt):
    """[kt*128, C] row-major -> [128, kt*C] partition-major (k-major cols)."""
    k128, c = mat.shape
    assert k128 == kt * 128
    return np.ascontiguousarray(
        mat.reshape(kt, 128, c).transpose(1, 0, 2).reshape(128, kt * c))


def kernel(x, router_w, router_b, w_up, w_down, shared_up, shared_down):
    global LAST_RESULTS, LAST_EXEC_NS
    x = np.asarray(x, dtype=np.float32)
    router_w = np.asarray(router_w, dtype=np.float32)
    router_b = np.asarray(router_b, dtype=np.float32)
    w_up = np.asarray(w_up, dtype=np.float32)
    w_down = np.asarray(w_down, dtype=np.float32)
    shared_up = np.asarray(shared_up, dtype=np.float32)
    shared_down = np.asarray(shared_down, dtype=np.float32)

    tidx, tw = _route_host(x, router_w, router_b)

    tok_of = [None] * E
    wgt_of = [None] * E
    for e in range(E):
        rows, cols = np.nonzero(tidx == e)
        tok_of[e] = rows
        wgt_of[e] = tw[rows, cols]
    counts = np.array([len(tok_of[e]) for e in range(E)])

    # bin-pack: rank groups of 8 per slot; greedy core assignment for balance
    order = np.argsort(-counts, kind="stable")
    assign = np.zeros((N_CORES, SLOTS), dtype=np.int64)
    core_load = np.zeros(N_CORES, dtype=np.int64)
    caps = []
    for j in range(SLOTS):
        grp = order[j * N_CORES:(j + 1) * N_CORES]
        caps.append(int(_roundup(max(int(counts[grp].max()), 16), 8)))
        cores_by_load = np.argsort(core_load, kind="stable")
        for i, e in enumerate(grp):  # grp is desc; pair big with least-loaded
            c = cores_by_load[i]
            assign[c, j] = e
            core_load[c] += counts[e]
    caps = tuple(caps)
    ntiles = [-(-c // 128) for c in caps]

    np_bf = mybir.dt.np(BF16)
    xt_full = np.ascontiguousarray(x.T).astype(np_bf)       # [H, T]
    su_cast = shared_up.astype(np_bf)
    sd_cast = shared_down.astype(np_bf)

    # shared-up packed m-major: [128, m*(KH*128) + k*128 + d]
    su_parts = []
    for r_tp in range(TP_S):
        blk = su_cast[:, r_tp * DS_LOC:(r_tp + 1) * DS_LOC]  # [H, DS_LOC]
        b3 = blk.reshape(KH, 128, MS, 128)  # [k, p, m, d]
        cols = b3.transpose(1, 2, 0, 3).reshape(128, MS * KH * 128)
        su_parts.append(np.ascontiguousarray(cols))
    sd_parts = [
        _pack_pm(sd_cast[r_tp * DS_LOC:(r_tp + 1) * DS_LOC, :], KS)
        for r_tp in range(TP_S)]
    xts_parts = [
        _pack_pm(xt_full[:, g * T_LOC:(g + 1) * T_LOC], KH)
        for g in range(DP_S)]

    in_maps = []
    for c in range(N_CORES):
        m = {}
        exp_ids = assign[c]
        wu_blocks = []
        wd_blocks = []
        for j in range(SLOTS):
            e = exp_ids[j]
            n = counts[e]
            cap = caps[j]
            # xt: [128, k*cap + c]
            xt_cj = np.zeros((H, cap), dtype=np_bf)
            xt_cj[:, :n] = xt_full[:, tok_of[e]]
            m[f"xt{j}"] = _pack_pm(xt_cj, KH)
            # cw: [128, ntiles]
            cw_cj = np.zeros((128 * ntiles[j],), dtype=np.float32)
            cw_cj[:n] = wgt_of[e]
            m[f"cw{j}"] = np.ascontiguousarray(
                cw_cj.reshape(ntiles[j], 128).T)
            # wu: consumption-order 128-col blocks (must match _up_layout)
            chunks_j, _, _ = _up_layout(cap)
            we = w_up[e].astype(np_bf)          # [H, DF]
            if len(chunks_j) == 1:
                # order (k, m): col = (k*8 + m)*128 + f
                wcols = we.reshape(KH, 128, MD, 128).transpose(
                    1, 0, 2, 3).reshape(128, KH * DF)
            else:
                # order (half, k, mi): col = (half*64 + k*4 + mi)*128 + f
                wcols = we.reshape(KH, 128, 2, MD // 2, 128).transpose(
                    1, 2, 0, 3, 4).reshape(128, KH * DF)
            wu_blocks.append(np.ascontiguousarray(wcols))
            # wd: [128, k2*H + h]
            wd_blocks.append(_pack_pm(w_down[e].astype(np_bf), KD))
        m["wu"] = np.ascontiguousarray(np.stack(wu_blocks))
        m["wd"] = np.ascontiguousarray(np.stack(wd_blocks))
        r_tp = c % TP_S
        g_dp = c // TP_S
        m["su"] = su_parts[r_tp]
        m["sd"] = sd_parts[r_tp]
        m["xts"] = xts_parts[g_dp]
        in_maps.append(m)

    key = (caps,)
    nc = _PROG_CACHE.get(key)
    if nc is None:
        nc = _build_program(caps)
        _PROG_CACHE[key] = nc

    res = run_bass_kernel_spmd(nc, in_maps, list(range(N_CORES)))
    LAST_RESULTS = res
    LAST_EXEC_NS = res.exec_time_ns

    out = np.zeros((T, H), dtype=np.float64)
    for c in range(N_CORES):
        g_dp = c // TP_S
        ys_c = res.results[c]["ys"].reshape(128, T_LOC // 128, H)
        ys_c = ys_c.transpose(1, 0, 2).reshape(T_LOC, H)
        out[g_dp * T_LOC:(g_dp + 1) * T_LOC] += ys_c.astype(np.float64)
        for j in range(SLOTS):
            e = assign[c, j]
            n = counts[e]
            if n:
                yr_c = res.results[c][f"yr{j}"].reshape(128, ntiles[j], H)
                yr_c = yr_c.transpose(1, 0, 2).reshape(ntiles[j] * 128, H)
                out[tok_of[e]] += yr_c[:n].astype(np.float64)
    return out.astype(np.float32)


# revision 11
# speedup vs baseline: 1.3360x; 1.0532x over previous
"""Trainium2 Bass kernel for NemotronH native MoE (T=2048, H=2048, E=32,
DF=1024, DS=4096, top-k=6, sigmoid router with group-limited routing).

Strategy (8 NeuronCores, full I/O):
  - Router + top-k run on host in fp32 numpy (bit-identical expert selection
    to the jax reference).
  - Expert parallelism: 32 routed experts bin-packed 4-per-core into 4
    "slots"; host gathers each expert's tokens into a transposed, padded
    activation block.  Slot capacities are baked into the Bass program
    (built per call, cached by capacity tuple).
  - All device inputs/outputs are host-prepacked into partition-major
    [128, X] layouts so each tensor moves with O(1) large DMAs instead of
    per-k-tile descriptors (DMA issue is ~0.7us/instruction on a queue).
  - Weight DMAs ride the gpsimd queue, activations the sync queue, outputs
    the scalar queue (same queue as the PSUM->SBUF copies that produce
    them, so no cross-queue head-of-line blocking).
  - GEMM loops are ordered so one LDWEIGHTS feeds multiple 512-wide
    matmuls (down/shared: k-outer, n-inner with 4 live PSUM banks), and
    consumption order follows the DMA arrival wave.
  - Down-projections consume a-tiles in production order (k2 == m), so
    up(j) -> down(j) chains with no PE bubble and the HAM clock gate never
    re-throttles mid-kernel.
  - Combine weights are folded into the PSUM->SBUF copy on the scalar
    engine (activation Copy with per-partition scale).
  - Shared expert: 4-way tensor-parallel over DS x 2-way data-parallel
    over tokens; partials summed on host.
  - Matmuls in bf16 (full-rate PE + FWL), fp32 PSUM accumulate, fp32 out.
"""

import sys
import numpy as np

try:
    import concourse.bacc as bacc  # noqa: F401
except ImportError:
    sys.path.insert(0, "/opt/trn_rl_repo")

import concourse.bacc as bacc
import concourse.tile as tile
from concourse import mybir
from concourse.bass_utils import run_bass_kernel_spmd

# ---- problem constants (hardcoded per contest rules) ----
T = 2048
H = 2048
E = 32
DF = 1024
DS = 4096
TOP_K = 6
N_GROUP = 8
TOPK_GROUP = 4
SCALE = 2.5
N_CORES = 8
SLOTS = 4          # routed experts per core
TP_S = 4           # shared expert: tensor-parallel degree over DS
DP_S = N_CORES // TP_S   # shared expert: token-parallel degree
DS_LOC = DS // TP_S      # 1024
T_LOC = T // DP_S        # 1024

KH = H // 128      # 16 k-tiles over H
MD = DF // 128     # 8 m-tiles over DF
KD = DF // 128     # 8 k-tiles over DF (down contraction)
NH = H // 512      # 4 n-chunks over H
MS = DS_LOC // 128  # 8 m-tiles over DS_LOC
KS = DS_LOC // 128  # 8 k-tiles over DS_LOC (shared down contraction)
NT = T_LOC // 512  # 2 token chunks (shared up rhs)

BF16 = mybir.dt.bfloat16
F32 = mybir.dt.float32

LAST_RESULTS = None
LAST_EXEC_NS = None

_PROG_CACHE = {}


def _route_host(x, router_w, router_b):
    """fp32 numpy replica of reference._route (bit-identical tidx)."""
    logits = x @ router_w.T
    scores = (1.0 / (1.0 + np.exp(-logits))).astype(np.float32)
    sfc = scores + router_b[None, :]
    gsize = E // N_GROUP
    grp = sfc.reshape(T, N_GROUP, gsize)
    g2 = -np.sort(-grp, axis=-1)[:, :, :2]
    group_scores = g2.sum(-1)
    gidx = np.argsort(-group_scores, axis=-1, kind="stable")[:, :TOPK_GROUP]
    group_mask = np.zeros((T, N_GROUP), dtype=sfc.dtype)
    np.put_along_axis(group_mask, gidx, 1.0, axis=1)
    score_mask = np.repeat(group_mask, gsize, axis=1)
    masked = np.where(score_mask > 0, sfc, 0.0)
    tidx = np.argsort(-masked, axis=-1, kind="stable")[:, :TOP_K].astype(np.int32)
    tw = np.take_along_axis(scores, tidx, axis=1)
    tw = tw / (tw.sum(-1, keepdims=True) + 1e-20)
    tw = (tw * SCALE).astype(np.float32)
    return tidx, tw


def _roundup(v, m):
    return -(-v // m) * m


def _up_chunks(cap):
    """Token chunks (<=512 wide) for the up-GEMM moving operand.

    Equal split: per-matmul overhead is small (~7ns), so two medium
    streams beat one 512 + one tiny remainder.
    """
    if cap <= 512:
        return [(0, cap)]
    assert cap <= 1024
    h1 = _roundup(cap // 2, 8)
    return [(0, h1), (h1, cap - h1)]


def _up_layout(cap):
    """(chunks, m_groups, block order) for the up pass of one slot.

    One PSUM bank per (m, chunk); the live set per m_group must be <= 8,
    so 2-chunk slots process DF m-tiles in halves.  The returned block
    order is the wu 128-col-block consumption order (shared with the host
    packer).
    """
    chunks = _up_chunks(cap)
    if len(chunks) == 1:
        m_groups = [list(range(MD))]
    else:
        m_groups = [list(range(0, MD // 2)), list(range(MD // 2, MD))]
    order = [(k, m) for mg in m_groups for k in range(KH) for m in mg]
    return chunks, m_groups, order


def _build_program(caps):
    ntiles = [-(-c // 128) for c in caps]
    nc = bacc.Bacc("TRN2", target_bir_lowering=False, debug=False,
                   num_devices=N_CORES)

    xt_r = [nc.dram_tensor(f"xt{j}", [128, KH * caps[j]], BF16,
                           kind="ExternalInput") for j in range(SLOTS)]
    cw_r = [nc.dram_tensor(f"cw{j}", [128, ntiles[j]], F32,
                           kind="ExternalInput") for j in range(SLOTS)]
    wu = nc.dram_tensor("wu", [SLOTS, 128, KH * DF], BF16,
                        kind="ExternalInput")
    wd = nc.dram_tensor("wd", [SLOTS, 128, KD * H], BF16,
                        kind="ExternalInput")
    su = nc.dram_tensor("su", [128, MS * KH * 128], BF16,
                        kind="ExternalInput")
    sd = nc.dram_tensor("sd", [128, KS * H], BF16, kind="ExternalInput")
    xts = nc.dram_tensor("xts", [128, KH * T_LOC], BF16,
                         kind="ExternalInput")
    yr = [nc.dram_tensor(f"yr{j}", [128, ntiles[j] * H], F32,
                         kind="ExternalOutput") for j in range(SLOTS)]
    ys = nc.dram_tensor("ys", [128, (T_LOC // 128) * H], F32,
                        kind="ExternalOutput")

    relu = mybir.ActivationFunctionType.Relu
    copyf = mybir.ActivationFunctionType.Copy
    CAPMAX = max(caps)

    with tile.TileContext(nc) as tc:
        with (
            tc.tile_pool(name="pp", bufs=8, space="PSUM") as pp,
            tc.tile_pool(name="xt", bufs=2) as xtp,        # [128,16*cap] bf16
            tc.tile_pool(name="wu", bufs=3) as wup,        # quarters, 8KB
            tc.tile_pool(name="wd", bufs=2) as wdp,        # halves, 8KB
            tc.tile_pool(name="su", bufs=6) as sup,        # m-blocks, 4KB
            tc.tile_pool(name="sx", bufs=1) as sxp,        # xts / sd, 32KB
            tc.tile_pool(name="as_", bufs=1) as asp,       # a_s, 16KB
            tc.tile_pool(name="at", bufs=2) as atp,        # a-tiles, 8.6KB
            tc.tile_pool(name="os", bufs=2) as osp,        # out stage, 8KB
            tc.tile_pool(name="rl", bufs=4) as rlp,        # relu tmp, 1KB
            tc.tile_pool(name="cw", bufs=4) as cwp,
        ):
            # All inputs ride ONE queue (sync) so transfers arrive at full
            # engine bandwidth in exactly the order compute consumes them.
            # Outputs ride the scalar queue (same queue as the copies that
            # produce them).
            NQ = 4                       # wu quarters per expert
            QW = KH * DF // NQ           # 4096 cols per quarter
            NWH = 2                      # wd halves per expert
            HW_ = KD * H // NWH          # 8192 cols per half

            def load_wu(j):
                qs = []
                for q in range(NQ):
                    t = wup.tile([128, QW], BF16, tag="wu", name=f"wu{j}_{q}")
                    nc.sync.dma_start(
                        t[:], wu.ap()[j, :, q * QW:(q + 1) * QW])
                    qs.append(t)
                return qs

            def load_wd(j):
                hs = []
                for hh in range(NWH):
                    t = wdp.tile([128, HW_], BF16, tag="wd", name=f"wd{j}_{hh}")
                    nc.sync.dma_start(
                        t[:], wd.ap()[j, :, hh * HW_:(hh + 1) * HW_])
                    hs.append(t)
                return hs

            def load_xt(j):
                t = xtp.tile([128, KH * caps[j]], BF16, tag="xt",
                             name=f"xt{j}")
                nc.sync.dma_start(t[:], xt_r[j].ap()[:, :])
                return t

            # program-order state
            a_t = {}
            wd_t = {}

            def emit_up(j, wu_q):
                cap = caps[j]
                chunks, m_groups, order = _up_layout(cap)
                bidx = {km: i for i, km in enumerate(order)}
                at_tile = atp.tile([128, MD * CAPMAX], BF16, tag="at",
                                   name=f"at{j}")
                for mg in m_groups:
                    ps = {}
                    for m in mg:
                        for ci in range(len(chunks)):
                            ps[(m, ci)] = pp.tile([128, 512], F32, tag="pp",
                                                  name=f"pu{j}_{m}_{ci}")
                    for k in range(KH):
                        for m in mg:
                            bi = bidx[(k, m)]
                            q, r = divmod(bi * 128, QW)
                            wsl = wu_q[q][:, r:r + 128]
                            for ci, (off, w) in enumerate(chunks):
                                nc.tensor.matmul(
                                    ps[(m, ci)][:, :w], wsl,
                                    xt_t[j][:, k * cap + off:k * cap + off + w],
                                    start=(k == 0), stop=(k == KH - 1))
                    for m in mg:
                        for ci, (off, w) in enumerate(chunks):
                            r = rlp.tile([128, 512], BF16, tag="rl",
                                         name=f"r{j}_{m}_{ci}")
                            nc.scalar.activation(r[:, :w], ps[(m, ci)][:, :w],
                                                 relu)
                            nc.vector.tensor_mul(
                                at_tile[:, m * cap + off:m * cap + off + w],
                                r[:, :w], r[:, :w])
                a_t[j] = at_tile
                del xt_t[j]

            def emit_down(j):
                cap = caps[j]
                at_tile = a_t.pop(j)
                wdh = wd_t.pop(j)
                for tci in range(ntiles[j]):
                    t0 = tci * 128
                    M = min(128, cap - t0)
                    ps = [pp.tile([128, 512], F32, tag="pp",
                                  name=f"pd{j}_{tci}_{nn}") for nn in range(NH)]
                    for k2 in range(KD):
                        hh, r = divmod(k2 * H, HW_)
                        asl = at_tile[:, k2 * cap + t0:k2 * cap + t0 + M]
                        for nn in range(NH):
                            nc.tensor.matmul(
                                ps[nn][:M, :], asl,
                                wdh[hh][:, r + nn * 512:r + (nn + 1) * 512],
                                start=(k2 == 0), stop=(k2 == KD - 1))
                    os_t = osp.tile([128, H], F32, tag="os",
                                    name=f"os{j}_{tci}")
                    for nn in range(NH):
                        nc.scalar.activation(
                            os_t[:M, nn * 512:(nn + 1) * 512], ps[nn][:M, :],
                            copyf, scale=cw_t[j][:M, tci:tci + 1])
                    nc.scalar.dma_start(yr[j].ap()[:M, tci * H:(tci + 1) * H],
                                        os_t[:M, :])

            # ================= schedule =================
            # Shared-up first: it needs the fewest input bytes (su m-block 0
            # + one xts quarter ≈ 1.5MB) so the PE starts ~4us in, and its
            # ~57us of compute covers the transfer of wu0/wd0/wu1.
            # DMA issue order on the sync queue == consumption order.
            su_t = [None] * MS
            su_t[0] = sup.tile([128, KH * 128], BF16, tag="su", name="su0")
            nc.sync.dma_start(su_t[0][:], su.ap()[:, 0:KH * 128])
            xts_t = sxp.tile([128, KH * T_LOC], BF16, tag="sx", name="xts")
            XQ = KH * T_LOC // 4
            for qq in range(4):
                nc.sync.dma_start(xts_t[:, qq * XQ:(qq + 1) * XQ],
                                  xts.ap()[:, qq * XQ:(qq + 1) * XQ])
            for m in range(1, MS):
                su_t[m] = sup.tile([128, KH * 128], BF16, tag="su",
                                   name=f"su{m}")
                nc.sync.dma_start(
                    su_t[m][:], su.ap()[:, m * (KH * 128):(m + 1) * (KH * 128)])
            xt_t = {0: load_xt(0)}
            wu_t = {0: load_wu(0)}
            wd_t[0] = load_wd(0)
            xt_t[1] = load_xt(1)
            wu_t[1] = load_wu(1)
            cw_t = []
            for j in range(SLOTS):
                t = cwp.tile([128, ntiles[j]], F32, tag="cw", name=f"cw{j}")
                nc.sync.dma_start(t[:], cw_r[j].ap()[:, :])
                cw_t.append(t)

            # ---------------- shared expert up ----------------
            # n-inner-of-m, k innermost: (m0, n0) needs only su block 0 and
            # the first xts quarter, so compute tracks the DMA wave.
            a_s = asp.tile([128, MS * T_LOC], BF16, tag="as", name="as")
            for m in range(MS):
                for n in range(NT):
                    ps = pp.tile([128, 512], F32, tag="pp", name=f"psu{m}_{n}")
                    for k in range(KH):
                        nc.tensor.matmul(
                            ps[:], su_t[m][:, k * 128:(k + 1) * 128],
                            xts_t[:, n * (KH * 512) + k * 512:
                                  n * (KH * 512) + (k + 1) * 512],
                            start=(k == 0), stop=(k == KH - 1))
                    r = rlp.tile([128, 512], BF16, tag="rl", name=f"rs{m}_{n}")
                    nc.scalar.activation(r[:], ps[:], relu)
                    nc.vector.tensor_mul(
                        a_s[:, m * T_LOC + n * 512:m * T_LOC + (n + 1) * 512],
                        r[:], r[:])

            # ---------------- routed experts ----------------
            for j in range(SLOTS):
                emit_up(j, wu_t.pop(j))
                if j + 2 < SLOTS:
                    xt_t[j + 2] = load_xt(j + 2)
                    wu_t[j + 2] = load_wu(j + 2)
                if j + 1 < SLOTS:
                    wd_t[j + 1] = load_wd(j + 1)
                if j == 2:
                    # sd reuses the xts buffer (freed at shared-up end); late
                    # emission avoids parking the queue on that reuse.
                    sd_t = sxp.tile([128, KS * H], BF16, tag="sx", name="sd")
                    nc.sync.dma_start(sd_t[:], sd.ap()[:, :])
                emit_down(j)

            # ---------------- shared expert down ----------------
            for tci in range(T_LOC // 128):
                t0 = tci * 128
                ps = [pp.tile([128, 512], F32, tag="pp", name=f"psd{tci}_{nn}")
                      for nn in range(NH)]
                for k2 in range(KS):
                    asl = a_s[:, k2 * T_LOC + t0:k2 * T_LOC + t0 + 128]
                    for nn in range(NH):
                        nc.tensor.matmul(
                            ps[nn][:], asl,
                            sd_t[:, k2 * H + nn * 512:k2 * H + (nn + 1) * 512],
                            start=(k2 == 0), stop=(k2 == KS - 1))
                os_t = osp.tile([128, H], F32, tag="os", name=f"oss{tci}")
                for nn in range(NH):
                    nc.scalar.activation(os_t[:, nn * 512:(nn + 1) * 512],
                                         ps[nn][:], copyf)
                nc.scalar.dma_start(ys.ap()[:, tci * H:(tci + 1) * H], os_t[:])

    nc.compile()
    return nc


def _pack_pm(mat, kt):
    """[kt*128, C] row-major -> [128, kt*C] partition-major (k-major cols)."""
    k128, c = mat.shape
    assert k128 == kt * 128
    return np.ascontiguousarray(
        mat.reshape(kt, 128, c).transpose(1, 0, 2).reshape(128, kt * c))


def kernel(x, router_w, router_b, w_up, w_down, shared_up, shared_down):
    global LAST_RESULTS, LAST_EXEC_NS
    x = np.asarray(x, dtype=np.float32)
    router_w = np.asarray(router_w, dtype=np.float32)
    router_b = np.asarray(router_b, dtype=np.float32)
    w_up = np.asarray(w_up, dtype=np.float32)
    w_down = np.asarray(w_down, dtype=np.float32)
    shared_up = np.asarray(shared_up, dtype=np.float32)
    shared_down = np.asarray(shared_down, dtype=np.float32)

    tidx, tw = _route_host(x, router_w, router_b)

    tok_of = [None] * E
    wgt_of = [None] * E
    for e in range(E):
        rows, cols = np.nonzero(tidx == e)
        tok_of[e] = rows
        wgt_of[e] = tw[rows, cols]
    counts = np.array([len(tok_of[e]) for e in range(E)])

    # bin-pack: rank groups of 8 per slot; greedy core assignment for balance
    order = np.argsort(-counts, kind="stable")
    assign = np.zeros((N_CORES, SLOTS), dtype=np.int64)
    core_load = np.zeros(N_CORES, dtype=np.int64)
    caps = []
    for j in range(SLOTS):
        grp = order[j * N_CORES:(j + 1) * N_CORES]
        caps.append(int(_roundup(max(int(counts[grp].max()), 16), 8)))
        cores_by_load = np.argsort(core_load, kind="stable")
        for i, e in enumerate(grp):  # grp is desc; pair big with least-loaded
            c = cores_by_load[i]
            assign[c, j] = e
            core_load[c] += counts[e]
    caps = tuple(caps)
    ntiles = [-(-c // 128) for c in caps]

    np_bf = mybir.dt.np(BF16)
    xt_full = np.ascontiguousarray(x.T).astype(np_bf)       # [H, T]
    su_cast = shared_up.astype(np_bf)
    sd_cast = shared_down.astype(np_bf)

    # shared-up packed m-major: [128, m*(KH*128) + k*128 + d]
    su_parts = []
    for r_tp in range(TP_S):
        blk = su_cast[:, r_tp * DS_LOC:(r_tp + 1) * DS_LOC]  # [H, DS_LOC]
        b3 = blk.reshape(KH, 128, MS, 128)  # [k, p, m, d]
        cols = b3.transpose(1, 2, 0, 3).reshape(128, MS * KH * 128)
        su_parts.append(np.ascontiguousarray(cols))
    sd_parts = [
        _pack_pm(sd_cast[r_tp * DS_LOC:(r_tp + 1) * DS_LOC, :], KS)
        for r_tp in range(TP_S)]
    # xts: n-chunk-major, [128, n*(KH*512) + k*512 + tt]
    xts_parts = []
    for g in range(DP_S):
        blocks = [
            _pack_pm(xt_full[:, g * T_LOC + n * 512:
                             g * T_LOC + (n + 1) * 512], KH)
            for n in range(NT)]
        xts_parts.append(np.ascontiguousarray(np.concatenate(blocks, axis=1)))

    in_maps = []
    for c in range(N_CORES):
        m = {}
        exp_ids = assign[c]
        wu_blocks = []
        wd_blocks = []
        for j in range(SLOTS):
            e = exp_ids[j]
            n = counts[e]
            cap = caps[j]
            # xt: [128, k*cap + c]
            xt_cj = np.zeros((H, cap), dtype=np_bf)
            xt_cj[:, :n] = xt_full[:, tok_of[e]]
            m[f"xt{j}"] = _pack_pm(xt_cj, KH)
            # cw: [128, ntiles]
            cw_cj = np.zeros((128 * ntiles[j],), dtype=np.float32)
            cw_cj[:n] = wgt_of[e]
            m[f"cw{j}"] = np.ascontiguousarray(
                cw_cj.reshape(ntiles[j], 128).T)
            # wu: consumption-order 128-col blocks (must match _up_layout)
            chunks_j, _, _ = _up_layout(cap)
            we = w_up[e].astype(np_bf)          # [H, DF]
            if len(chunks_j) == 1:
                # order (k, m): col = (k*8 + m)*128 + f
                wcols = we.reshape(KH, 128, MD, 128).transpose(
                    1, 0, 2, 3).reshape(128, KH * DF)
            else:
                # order (half, k, mi): col = (half*64 + k*4 + mi)*128 + f
                wcols = we.reshape(KH, 128, 2, MD // 2, 128).transpose(
                    1, 2, 0, 3, 4).reshape(128, KH * DF)
            wu_blocks.append(np.ascontiguousarray(wcols))
            # wd: [128, k2*H + h]
            wd_blocks.append(_pack_pm(w_down[e].astype(np_bf), KD))
        m["wu"] = np.ascontiguousarray(np.stack(wu_blocks))
        m["wd"] = np.ascontiguousarray(np.stack(wd_blocks))
        r_tp = c % TP_S
        g_dp = c // TP_S
        m["su"] = su_parts[r_tp]
        m["sd"] = sd_parts[r_tp]
        m["xts"] = xts_parts[g_dp]
        in_maps.append(m)

    key = (caps,)
    nc = _PROG_CACHE.get(key)
    if nc is None:
        nc = _build_program(caps)
        _PROG_CACHE[key] = nc

    res = run_bass_kernel_spmd(nc, in_maps, list(range(N_CORES)))
    LAST_RESULTS = res
    LAST_EXEC_NS = res.exec_time_ns

    out = np.zeros((T, H), dtype=np.float64)
    for c in range(N_CORES):
        g_dp = c // TP_S
        ys_c = res.results[c]["ys"].reshape(128, T_LOC // 128, H)
        ys_c = ys_c.transpose(1, 0, 2).reshape(T_LOC, H)
        out[g_dp * T_LOC:(g_dp + 1) * T_LOC] += ys_c.astype(np.float64)
        for j in range(SLOTS):
            e = assign[c, j]
            n = counts[e]
            if n:
                yr_c = res.results[c][f"yr{j}"].reshape(128, ntiles[j], H)
                yr_c = yr_c.transpose(1, 0, 2).reshape(ntiles[j] * 128, H)
                out[tok_of[e]] += yr_c[:n].astype(np.float64)
    return out.astype(np.float32)


# revision 17
# speedup vs baseline: 1.3487x; 1.0095x over previous
"""Trainium2 Bass kernel for NemotronH native MoE (T=2048, H=2048, E=32,
DF=1024, DS=4096, top-k=6, sigmoid router with group-limited routing).

Strategy (8 NeuronCores, full I/O):
  - Router + top-k run on host in fp32 numpy (bit-identical expert selection
    to the jax reference).
  - Expert parallelism: 32 routed experts bin-packed 4-per-core into 4
    "slots"; host gathers each expert's tokens into a transposed, padded
    activation block.  Slot capacities are baked into the Bass program
    (built per call, cached by capacity tuple).
  - All device inputs/outputs are host-prepacked into partition-major
    [128, X] layouts so each tensor moves with O(1) large DMAs instead of
    per-k-tile descriptors (DMA issue is ~0.7us/instruction on a queue).
  - Weight DMAs ride the gpsimd queue, activations the sync queue, outputs
    the scalar queue (same queue as the PSUM->SBUF copies that produce
    them, so no cross-queue head-of-line blocking).
  - GEMM loops are ordered so one LDWEIGHTS feeds multiple 512-wide
    matmuls (down/shared: k-outer, n-inner with 4 live PSUM banks), and
    consumption order follows the DMA arrival wave.
  - Down-projections consume a-tiles in production order (k2 == m), so
    up(j) -> down(j) chains with no PE bubble and the HAM clock gate never
    re-throttles mid-kernel.
  - Combine weights are folded into the PSUM->SBUF copy on the scalar
    engine (activation Copy with per-partition scale).
  - Shared expert: 4-way tensor-parallel over DS x 2-way data-parallel
    over tokens; partials summed on host.
  - Matmuls in bf16 (full-rate PE + FWL), fp32 PSUM accumulate, fp32 out.
"""

import sys
import numpy as np

try:
    import concourse.bacc as bacc  # noqa: F401
except ImportError:
    sys.path.insert(0, "/opt/trn_rl_repo")

import concourse.bacc as bacc
import concourse.tile as tile
from concourse import mybir
from concourse.bass_utils import run_bass_kernel_spmd

# ---- problem constants (hardcoded per contest rules) ----
T = 2048
H = 2048
E = 32
DF = 1024
DS = 4096
TOP_K = 6
N_GROUP = 8
TOPK_GROUP = 4
SCALE = 2.5
N_CORES = 8
SLOTS = 4          # routed experts per core
TP_S = 4           # shared expert: tensor-parallel degree over DS
DP_S = N_CORES // TP_S   # shared expert: token-parallel degree
DS_LOC = DS // TP_S      # 1024
T_LOC = T // DP_S        # 1024

KH = H // 128      # 16 k-tiles over H
MD = DF // 128     # 8 m-tiles over DF
KD = DF // 128     # 8 k-tiles over DF (down contraction)
NH = H // 512      # 4 n-chunks over H
MS = DS_LOC // 128  # 8 m-tiles over DS_LOC
KS = DS_LOC // 128  # 8 k-tiles over DS_LOC (shared down contraction)
NT = T_LOC // 512  # 2 token chunks (shared up rhs)

BF16 = mybir.dt.bfloat16
F32 = mybir.dt.float32

LAST_RESULTS = None
LAST_EXEC_NS = None

_PROG_CACHE = {}


def _route_host(x, router_w, router_b):
    """fp32 numpy replica of reference._route (bit-identical tidx)."""
    logits = x @ router_w.T
    scores = (1.0 / (1.0 + np.exp(-logits))).astype(np.float32)
    sfc = scores + router_b[None, :]
    gsize = E // N_GROUP
    grp = sfc.reshape(T, N_GROUP, gsize)
    g2 = -np.sort(-grp, axis=-1)[:, :, :2]
    group_scores = g2.sum(-1)
    gidx = np.argsort(-group_scores, axis=-1, kind="stable")[:, :TOPK_GROUP]
    group_mask = np.zeros((T, N_GROUP), dtype=sfc.dtype)
    np.put_along_axis(group_mask, gidx, 1.0, axis=1)
    score_mask = np.repeat(group_mask, gsize, axis=1)
    masked = np.where(score_mask > 0, sfc, 0.0)
    tidx = np.argsort(-masked, axis=-1, kind="stable")[:, :TOP_K].astype(np.int32)
    tw = np.take_along_axis(scores, tidx, axis=1)
    tw = tw / (tw.sum(-1, keepdims=True) + 1e-20)
    tw = (tw * SCALE).astype(np.float32)
    return tidx, tw


def _roundup(v, m):
    return -(-v // m) * m


def _up_chunks(cap):
    """Token chunks (<=512 wide) for the up-GEMM moving operand.

    Equal split: per-matmul overhead is small (~7ns), so two medium
    streams beat one 512 + one tiny remainder.
    """
    if cap <= 512:
        return [(0, cap)]
    assert cap <= 1024
    h1 = _roundup(cap // 2, 8)
    return [(0, h1), (h1, cap - h1)]


def _up_layout(cap):
    """(chunks, m_groups, block order) for the up pass of one slot.

    One PSUM bank per (m, chunk); the live set per m_group must be <= 8,
    so 2-chunk slots process DF m-tiles in halves.  The returned block
    order is the wu 128-col-block consumption order (shared with the host
    packer).
    """
    chunks = _up_chunks(cap)
    if len(chunks) == 1:
        m_groups = [list(range(MD))]
    else:
        m_groups = [list(range(0, MD // 2)), list(range(MD // 2, MD))]
    order = [(k, m) for mg in m_groups for k in range(KH) for m in mg]
    return chunks, m_groups, order


def _build_program(caps):
    ntiles = [-(-c // 128) for c in caps]
    nc = bacc.Bacc("TRN2", target_bir_lowering=False, debug=False,
                   num_devices=N_CORES)

    xt_r = [nc.dram_tensor(f"xt{j}", [128, KH * caps[j]], BF16,
                           kind="ExternalInput") for j in range(SLOTS)]
    cw_r = [nc.dram_tensor(f"cw{j}", [128, ntiles[j]], F32,
                           kind="ExternalInput") for j in range(SLOTS)]
    wu = nc.dram_tensor("wu", [SLOTS, 128, KH * DF], BF16,
                        kind="ExternalInput")
    wd = nc.dram_tensor("wd", [SLOTS, 128, KD * H], BF16,
                        kind="ExternalInput")
    su = nc.dram_tensor("su", [128, MS * KH * 128], BF16,
                        kind="ExternalInput")
    sd = nc.dram_tensor("sd", [128, KS * H], BF16, kind="ExternalInput")
    xts = nc.dram_tensor("xts", [128, KH * T_LOC], BF16,
                         kind="ExternalInput")
    yr = [nc.dram_tensor(f"yr{j}", [128, ntiles[j] * H], F32,
                         kind="ExternalOutput") for j in range(SLOTS)]
    ys = nc.dram_tensor("ys", [128, (T_LOC // 128) * H], F32,
                        kind="ExternalOutput")

    relu = mybir.ActivationFunctionType.Relu
    copyf = mybir.ActivationFunctionType.Copy
    CAPMAX = max(caps)

    with tile.TileContext(nc) as tc:
        with (
            tc.tile_pool(name="pp", bufs=8, space="PSUM") as pp,
            tc.tile_pool(name="xt", bufs=2) as xtp,        # [128,16*cap] bf16
            tc.tile_pool(name="wu", bufs=3) as wup,        # quarters, 8KB
            tc.tile_pool(name="wd", bufs=2) as wdp,        # halves, 8KB
            tc.tile_pool(name="su", bufs=6) as sup,        # m-blocks, 4KB
            tc.tile_pool(name="sx", bufs=1) as sxp,        # xts / sd, 32KB
            tc.tile_pool(name="as_", bufs=1) as asp,       # a_s, 16KB
            tc.tile_pool(name="at", bufs=2) as atp,        # a-tiles, 8.6KB
            tc.tile_pool(name="os", bufs=2) as osp,        # out stage, 8KB
            tc.tile_pool(name="rl", bufs=4) as rlp,        # relu tmp, 1KB
            tc.tile_pool(name="cw", bufs=4) as cwp,
        ):
            # All inputs ride ONE queue (sync) so transfers arrive at full
            # engine bandwidth in exactly the order compute consumes them.
            # Outputs ride the scalar queue (same queue as the copies that
            # produce them).
            NQ = 4                       # wu quarters per expert
            QW = KH * DF // NQ           # 4096 cols per quarter
            NWH = 2                      # wd halves per expert
            HW_ = KD * H // NWH          # 8192 cols per half

            def load_wu(j):
                qs = []
                for q in range(NQ):
                    t = wup.tile([128, QW], BF16, tag="wu", name=f"wu{j}_{q}")
                    nc.sync.dma_start(
                        t[:], wu.ap()[j, :, q * QW:(q + 1) * QW])
                    qs.append(t)
                return qs

            def load_wd(j):
                hs = []
                for hh in range(NWH):
                    t = wdp.tile([128, HW_], BF16, tag="wd", name=f"wd{j}_{hh}")
                    nc.sync.dma_start(
                        t[:], wd.ap()[j, :, hh * HW_:(hh + 1) * HW_])
                    hs.append(t)
                return hs

            def load_xt(j):
                t = xtp.tile([128, KH * caps[j]], BF16, tag="xt",
                             name=f"xt{j}")
                nc.sync.dma_start(t[:], xt_r[j].ap()[:, :])
                return t

            # program-order state
            a_t = {}
            wd_t = {}

            def emit_up(j, wu_q):
                cap = caps[j]
                chunks, m_groups, order = _up_layout(cap)
                bidx = {km: i for i, km in enumerate(order)}
                at_tile = atp.tile([128, MD * CAPMAX], BF16, tag="at",
                                   name=f"at{j}")
                for mg in m_groups:
                    ps = {}
                    for m in mg:
                        for ci in range(len(chunks)):
                            ps[(m, ci)] = pp.tile([128, 512], F32, tag="pp",
                                                  name=f"pu{j}_{m}_{ci}")
                    for k in range(KH):
                        for m in mg:
                            bi = bidx[(k, m)]
                            q, r = divmod(bi * 128, QW)
                            wsl = wu_q[q][:, r:r + 128]
                            for ci, (off, w) in enumerate(chunks):
                                nc.tensor.matmul(
                                    ps[(m, ci)][:, :w], wsl,
                                    xt_t[j][:, k * cap + off:k * cap + off + w],
                                    start=(k == 0), stop=(k == KH - 1))
                    for m in mg:
                        for ci, (off, w) in enumerate(chunks):
                            r = rlp.tile([128, 512], BF16, tag="rl",
                                         name=f"r{j}_{m}_{ci}")
                            nc.scalar.activation(r[:, :w], ps[(m, ci)][:, :w],
                                                 relu)
                            nc.vector.tensor_mul(
                                at_tile[:, m * cap + off:m * cap + off + w],
                                r[:, :w], r[:, :w])
                a_t[j] = at_tile
                del xt_t[j]

            def emit_down(j):
                cap = caps[j]
                at_tile = a_t.pop(j)
                wdh = wd_t.pop(j)
                for tci in range(ntiles[j]):
                    t0 = tci * 128
                    M = min(128, cap - t0)
                    ps = [pp.tile([128, 512], F32, tag="pp",
                                  name=f"pd{j}_{tci}_{nn}") for nn in range(NH)]
                    for k2 in range(KD):
                        hh, r = divmod(k2 * H, HW_)
                        asl = at_tile[:, k2 * cap + t0:k2 * cap + t0 + M]
                        for nn in range(NH):
                            nc.tensor.matmul(
                                ps[nn][:M, :], asl,
                                wdh[hh][:, r + nn * 512:r + (nn + 1) * 512],
                                start=(k2 == 0), stop=(k2 == KD - 1))
                    # psum->sbuf copies (with combine-weight scale) split
                    # scalar/vector; each half DMAs from its own queue.
                    os_t = osp.tile([128, H], F32, tag="os",
                                    name=f"os{j}_{tci}")
                    csl = cw_t[j][:M, tci:tci + 1]
                    for nn in range(2):
                        nc.scalar.activation(
                            os_t[:M, nn * 512:(nn + 1) * 512], ps[nn][:M, :],
                            copyf, scale=csl)
                    nc.scalar.dma_start(
                        yr[j].ap()[:M, tci * H:tci * H + 1024], os_t[:M, :1024])
                    for nn in range(2, NH):
                        nc.vector.tensor_scalar_mul(
                            os_t[:M, nn * 512:(nn + 1) * 512], ps[nn][:M, :],
                            csl)
                    nc.gpsimd.dma_start(
                        yr[j].ap()[:M, tci * H + 1024:(tci + 1) * H],
                        os_t[:M, 1024:])

            # ================= schedule =================
            # Shared-up first: it needs the fewest input bytes (su m-block 0
            # + one xts quarter ≈ 1.5MB) so the PE starts ~4us in, and its
            # ~57us of compute covers the transfer of wu0/wd0/wu1.
            # DMA issue order on the sync queue == consumption order.
            su_t = [None] * MS
            su_t[0] = sup.tile([128, KH * 128], BF16, tag="su", name="su0")
            nc.sync.dma_start(su_t[0][:], su.ap()[:, 0:KH * 128])
            xts_t = sxp.tile([128, KH * T_LOC], BF16, tag="sx", name="xts")
            XQ = KH * T_LOC // 4
            for qq in range(4):
                nc.sync.dma_start(xts_t[:, qq * XQ:(qq + 1) * XQ],
                                  xts.ap()[:, qq * XQ:(qq + 1) * XQ])
            for m in range(1, MS):
                su_t[m] = sup.tile([128, KH * 128], BF16, tag="su",
                                   name=f"su{m}")
                nc.sync.dma_start(
                    su_t[m][:], su.ap()[:, m * (KH * 128):(m + 1) * (KH * 128)])
            xt_t = {0: load_xt(0)}
            wu_t = {0: load_wu(0)}
            wd_t[0] = load_wd(0)
            xt_t[1] = load_xt(1)
            wu_t[1] = load_wu(1)
            cw_t = []
            for j in range(SLOTS):
                t = cwp.tile([128, ntiles[j]], F32, tag="cw", name=f"cw{j}")
                nc.sync.dma_start(t[:], cw_r[j].ap()[:, :])
                cw_t.append(t)

            # ---------------- shared expert up ----------------
            # n-inner-of-m, k innermost: (m0, n0) needs only su block 0 and
            # the first xts quarter, so compute tracks the DMA wave.
            a_s = asp.tile([128, MS * T_LOC], BF16, tag="as", name="as")
            for m in range(MS):
                for n in range(NT):
                    ps = pp.tile([128, 512], F32, tag="pp", name=f"psu{m}_{n}")
                    for k in range(KH):
                        nc.tensor.matmul(
                            ps[:], su_t[m][:, k * 128:(k + 1) * 128],
                            xts_t[:, n * (KH * 512) + k * 512:
                                  n * (KH * 512) + (k + 1) * 512],
                            start=(k == 0), stop=(k == KH - 1))
                    r = rlp.tile([128, 512], BF16, tag="rl", name=f"rs{m}_{n}")
                    nc.scalar.activation(r[:], ps[:], relu)
                    nc.vector.tensor_mul(
                        a_s[:, m * T_LOC + n * 512:m * T_LOC + (n + 1) * 512],
                        r[:], r[:])

            # ---------------- routed experts ----------------
            # wd(j+1) is emitted before the xt/wu(j+2) prefetches: the
            # latter park the queue on ring-buffer reuse, and wd must not
            # queue behind that park.
            for j in range(SLOTS):
                emit_up(j, wu_t.pop(j))
                if j + 1 < SLOTS:
                    wd_t[j + 1] = load_wd(j + 1)
                if j + 2 < SLOTS:
                    xt_t[j + 2] = load_xt(j + 2)
                    wu_t[j + 2] = load_wu(j + 2)
                if j == 2:
                    # sd reuses the xts buffer (freed at shared-up end); late
                    # emission avoids parking the queue on that reuse.
                    sd_t = sxp.tile([128, KS * H], BF16, tag="sx", name="sd")
                    nc.sync.dma_start(sd_t[:], sd.ap()[:, :])
                emit_down(j)

            # ---------------- shared expert down ----------------
            for tci in range(T_LOC // 128):
                t0 = tci * 128
                ps = [pp.tile([128, 512], F32, tag="pp", name=f"psd{tci}_{nn}")
                      for nn in range(NH)]
                for k2 in range(KS):
                    asl = a_s[:, k2 * T_LOC + t0:k2 * T_LOC + t0 + 128]
                    for nn in range(NH):
                        nc.tensor.matmul(
                            ps[nn][:], asl,
                            sd_t[:, k2 * H + nn * 512:k2 * H + (nn + 1) * 512],
                            start=(k2 == 0), stop=(k2 == KS - 1))
                os_t = osp.tile([128, H], F32, tag="os", name=f"oss{tci}")
                for nn in range(2):
                    nc.scalar.activation(os_t[:, nn * 512:(nn + 1) * 512],
                                         ps[nn][:], copyf)
                nc.scalar.dma_start(ys.ap()[:, tci * H:tci * H + 1024],
                                    os_t[:, :1024])
                for nn in range(2, NH):
                    nc.vector.tensor_copy(os_t[:, nn * 512:(nn + 1) * 512],
                                          ps[nn][:])
                nc.gpsimd.dma_start(ys.ap()[:, tci * H + 1024:(tci + 1) * H],
                                    os_t[:, 1024:])

    nc.compile()
    return nc


def _pack_pm(mat, kt):
    """[kt*128, C] row-major -> [128, kt*C] partition-major (k-major cols)."""
    k128, c = mat.shape
    assert k128 == kt * 128
    return np.ascontiguousarray(
        mat.reshape(kt, 128, c).transpose(1, 0, 2).reshape(128, kt * c))


def kernel(x, router_w, router_b, w_up, w_down, shared_up, shared_down):
    global LAST_RESULTS, LAST_EXEC_NS
    x = np.asarray(x, dtype=np.float32)
    router_w = np.asarray(router_w, dtype=np.float32)
    router_b = np.asarray(router_b, dtype=np.float32)
    w_up = np.asarray(w_up, dtype=np.float32)
    w_down = np.asarray(w_down, dtype=np.float32)
    shared_up = np.asarray(shared_up, dtype=np.float32)
    shared_down = np.asarray(shared_down, dtype=np.float32)

    tidx, tw = _route_host(x, router_w, router_b)

    tok_of = [None] * E
    wgt_of = [None] * E
    for e in range(E):
        rows, cols = np.nonzero(tidx == e)
        tok_of[e] = rows
        wgt_of[e] = tw[rows, cols]
    counts = np.array([len(tok_of[e]) for e in range(E)])

    # bin-pack: rank groups of 8 per slot; greedy core assignment for balance
    order = np.argsort(-counts, kind="stable")
    assign = np.zeros((N_CORES, SLOTS), dtype=np.int64)
    core_load = np.zeros(N_CORES, dtype=np.int64)
    caps = []
    for j in range(SLOTS):
        grp = order[j * N_CORES:(j + 1) * N_CORES]
        caps.append(int(_roundup(max(int(counts[grp].max()), 16), 8)))
        cores_by_load = np.argsort(core_load, kind="stable")
        for i, e in enumerate(grp):  # grp is desc; pair big with least-loaded
            c = cores_by_load[i]
            assign[c, j] = e
            core_load[c] += counts[e]
    caps = tuple(caps)
    ntiles = [-(-c // 128) for c in caps]

    np_bf = mybir.dt.np(BF16)
    xt_full = np.ascontiguousarray(x.T).astype(np_bf)       # [H, T]
    su_cast = shared_up.astype(np_bf)
    sd_cast = shared_down.astype(np_bf)

    # shared-up packed m-major: [128, m*(KH*128) + k*128 + d]
    su_parts = []
    for r_tp in range(TP_S):
        blk = su_cast[:, r_tp * DS_LOC:(r_tp + 1) * DS_LOC]  # [H, DS_LOC]
        b3 = blk.reshape(KH, 128, MS, 128)  # [k, p, m, d]
        cols = b3.transpose(1, 2, 0, 3).reshape(128, MS * KH * 128)
        su_parts.append(np.ascontiguousarray(cols))
    sd_parts = [
        _pack_pm(sd_cast[r_tp * DS_LOC:(r_tp + 1) * DS_LOC, :], KS)
        for r_tp in range(TP_S)]
    # xts: n-chunk-major, [128, n*(KH*512) + k*512 + tt]
    xts_parts = []
    for g in range(DP_S):
        blocks = [
            _pack_pm(xt_full[:, g * T_LOC + n * 512:
                             g * T_LOC + (n + 1) * 512], KH)
            for n in range(NT)]
        xts_parts.append(np.ascontiguousarray(np.concatenate(blocks, axis=1)))

    in_maps = []
    for c in range(N_CORES):
        m = {}
        exp_ids = assign[c]
        wu_blocks = []
        wd_blocks = []
        for j in range(SLOTS):
            e = exp_ids[j]
            n = counts[e]
            cap = caps[j]
            # xt: [128, k*cap + c]
            xt_cj = np.zeros((H, cap), dtype=np_bf)
            xt_cj[:, :n] = xt_full[:, tok_of[e]]
            m[f"xt{j}"] = _pack_pm(xt_cj, KH)
            # cw: [128, ntiles]
            cw_cj = np.zeros((128 * ntiles[j],), dtype=np.float32)
            cw_cj[:n] = wgt_of[e]
            m[f"cw{j}"] = np.ascontiguousarray(
                cw_cj.reshape(ntiles[j], 128).T)
            # wu: consumption-order 128-col blocks (must match _up_layout)
            chunks_j, _, _ = _up_layout(cap)
            we = w_up[e].astype(np_bf)          # [H, DF]
            if len(chunks_j) == 1:
                # order (k, m): col = (k*8 + m)*128 + f
                wcols = we.reshape(KH, 128, MD, 128).transpose(
                    1, 0, 2, 3).reshape(128, KH * DF)
            else:
                # order (half, k, mi): col = (half*64 + k*4 + mi)*128 + f
                wcols = we.reshape(KH, 128, 2, MD // 2, 128).transpose(
                    1, 2, 0, 3, 4).reshape(128, KH * DF)
            wu_blocks.append(np.ascontiguousarray(wcols))
            # wd: [128, k2*H + h]
            wd_blocks.append(_pack_pm(w_down[e].astype(np_bf), KD))
        m["wu"] = np.ascontiguousarray(np.stack(wu_blocks))
        m["wd"] = np.ascontiguousarray(np.stack(wd_blocks))
        r_tp = c % TP_S
        g_dp = c // TP_S
        m["su"] = su_parts[r_tp]
        m["sd"] = sd_parts[r_tp]
        m["xts"] = xts_parts[g_dp]
        in_maps.append(m)

    key = (caps,)
    nc = _PROG_CACHE.get(key)
    if nc is None:
        nc = _build_program(caps)
        _PROG_CACHE[key] = nc

    res = run_bass_kernel_spmd(nc, in_maps, list(range(N_CORES)))
    LAST_RESULTS = res
    LAST_EXEC_NS = res.exec_time_ns

    out = np.zeros((T, H), dtype=np.float64)
    for c in range(N_CORES):
        g_dp = c // TP_S
        ys_c = res.results[c]["ys"].reshape(128, T_LOC // 128, H)
        ys_c = ys_c.transpose(1, 0, 2).reshape(T_LOC, H)
        out[g_dp * T_LOC:(g_dp + 1) * T_LOC] += ys_c.astype(np.float64)
        for j in range(SLOTS):
            e = assign[c, j]
            n = counts[e]
            if n:
                yr_c = res.results[c][f"yr{j}"].reshape(128, ntiles[j], H)
                yr_c = yr_c.transpose(1, 0, 2).reshape(ntiles[j] * 128, H)
                out[tok_of[e]] += yr_c[:n].astype(np.float64)
    return out.astype(np.float32)


# revision 20
# speedup vs baseline: 1.3519x; 1.0024x over previous
"""Trainium2 Bass kernel for NemotronH native MoE (T=2048, H=2048, E=32,
DF=1024, DS=4096, top-k=6, sigmoid router with group-limited routing).

Strategy (8 NeuronCores, full I/O):
  - Router + top-k run on host in fp32 numpy (bit-identical expert selection
    to the jax reference).
  - Expert parallelism: 32 routed experts bin-packed 4-per-core into 4
    "slots"; host gathers each expert's tokens into a transposed, padded
    activation block.  Slot capacities are baked into the Bass program
    (built per call, cached by capacity tuple).
  - All device inputs/outputs are host-prepacked into partition-major
    [128, X] layouts so each tensor moves with O(1) large DMAs instead of
    per-k-tile descriptors (DMA issue is ~0.7us/instruction on a queue).
  - Weight DMAs ride the gpsimd queue, activations the sync queue, outputs
    the scalar queue (same queue as the PSUM->SBUF copies that produce
    them, so no cross-queue head-of-line blocking).
  - GEMM loops are ordered so one LDWEIGHTS feeds multiple 512-wide
    matmuls (down/shared: k-outer, n-inner with 4 live PSUM banks), and
    consumption order follows the DMA arrival wave.
  - Down-projections consume a-tiles in production order (k2 == m), so
    up(j) -> down(j) chains with no PE bubble and the HAM clock gate never
    re-throttles mid-kernel.
  - Combine weights are folded into the PSUM->SBUF copy on the scalar
    engine (activation Copy with per-partition scale).
  - Shared expert: 4-way tensor-parallel over DS x 2-way data-parallel
    over tokens; partials summed on host.
  - Matmuls in bf16 (full-rate PE + FWL), fp32 PSUM accumulate, fp32 out.
"""

import sys
import numpy as np

try:
    import concourse.bacc as bacc  # noqa: F401
except ImportError:
    sys.path.insert(0, "/opt/trn_rl_repo")

import concourse.bacc as bacc
import concourse.tile as tile
from concourse import mybir
from concourse.bass_utils import run_bass_kernel_spmd

# ---- problem constants (hardcoded per contest rules) ----
T = 2048
H = 2048
E = 32
DF = 1024
DS = 4096
TOP_K = 6
N_GROUP = 8
TOPK_GROUP = 4
SCALE = 2.5
N_CORES = 8
SLOTS = 4          # routed experts per core
TP_S = 4           # shared expert: tensor-parallel degree over DS
DP_S = N_CORES // TP_S   # shared expert: token-parallel degree
DS_LOC = DS // TP_S      # 1024
T_LOC = T // DP_S        # 1024

KH = H // 128      # 16 k-tiles over H
MD = DF // 128     # 8 m-tiles over DF
KD = DF // 128     # 8 k-tiles over DF (down contraction)
NH = H // 512      # 4 n-chunks over H
MS = DS_LOC // 128  # 8 m-tiles over DS_LOC
KS = DS_LOC // 128  # 8 k-tiles over DS_LOC (shared down contraction)
NT = T_LOC // 512  # 2 token chunks (shared up rhs)

BF16 = mybir.dt.bfloat16
F32 = mybir.dt.float32

LAST_RESULTS = None
LAST_EXEC_NS = None

_PROG_CACHE = {}


def _route_host(x, router_w, router_b):
    """fp32 numpy replica of reference._route (bit-identical tidx)."""
    logits = x @ router_w.T
    scores = (1.0 / (1.0 + np.exp(-logits))).astype(np.float32)
    sfc = scores + router_b[None, :]
    gsize = E // N_GROUP
    grp = sfc.reshape(T, N_GROUP, gsize)
    g2 = -np.sort(-grp, axis=-1)[:, :, :2]
    group_scores = g2.sum(-1)
    gidx = np.argsort(-group_scores, axis=-1, kind="stable")[:, :TOPK_GROUP]
    group_mask = np.zeros((T, N_GROUP), dtype=sfc.dtype)
    np.put_along_axis(group_mask, gidx, 1.0, axis=1)
    score_mask = np.repeat(group_mask, gsize, axis=1)
    masked = np.where(score_mask > 0, sfc, 0.0)
    tidx = np.argsort(-masked, axis=-1, kind="stable")[:, :TOP_K].astype(np.int32)
    tw = np.take_along_axis(scores, tidx, axis=1)
    tw = tw / (tw.sum(-1, keepdims=True) + 1e-20)
    tw = (tw * SCALE).astype(np.float32)
    return tidx, tw


def _roundup(v, m):
    return -(-v // m) * m


def _up_chunks(cap):
    """Token chunks (<=512 wide) for the up-GEMM moving operand.

    Equal split: per-matmul overhead is small (~7ns), so two medium
    streams beat one 512 + one tiny remainder.
    """
    if cap <= 512:
        return [(0, cap)]
    assert cap <= 1024
    h1 = _roundup(cap // 2, 8)
    return [(0, h1), (h1, cap - h1)]


def _up_layout(cap):
    """(chunks, m_groups, block order) for the up pass of one slot.

    One PSUM bank per (m, chunk); the live set per m_group must be <= 8,
    so 2-chunk slots process DF m-tiles in halves.  The returned block
    order is the wu 128-col-block consumption order (shared with the host
    packer).
    """
    chunks = _up_chunks(cap)
    if len(chunks) == 1:
        m_groups = [list(range(MD))]
    else:
        m_groups = [list(range(0, MD // 2)), list(range(MD // 2, MD))]
    order = [(k, m) for mg in m_groups for k in range(KH) for m in mg]
    return chunks, m_groups, order


def _build_program(caps):
    ntiles = [-(-c // 128) for c in caps]
    nc = bacc.Bacc("TRN2", target_bir_lowering=False, debug=False,
                   num_devices=N_CORES)

    xt_r = [nc.dram_tensor(f"xt{j}", [128, KH * caps[j]], BF16,
                           kind="ExternalInput") for j in range(SLOTS)]
    cw_r = [nc.dram_tensor(f"cw{j}", [128, ntiles[j]], F32,
                           kind="ExternalInput") for j in range(SLOTS)]
    wu = nc.dram_tensor("wu", [SLOTS, 128, KH * DF], BF16,
                        kind="ExternalInput")
    wd = nc.dram_tensor("wd", [SLOTS, 128, KD * H], BF16,
                        kind="ExternalInput")
    su = nc.dram_tensor("su", [128, MS * KH * 128], BF16,
                        kind="ExternalInput")
    sd = nc.dram_tensor("sd", [128, KS * H], BF16, kind="ExternalInput")
    xts = nc.dram_tensor("xts", [128, KH * T_LOC], BF16,
                         kind="ExternalInput")
    yr = [nc.dram_tensor(f"yr{j}", [128, ntiles[j] * H], F32,
                         kind="ExternalOutput") for j in range(SLOTS)]
    ys = nc.dram_tensor("ys", [128, (T_LOC // 128) * H], F32,
                        kind="ExternalOutput")

    relu = mybir.ActivationFunctionType.Relu
    copyf = mybir.ActivationFunctionType.Copy
    CAPMAX = max(caps)

    with tile.TileContext(nc) as tc:
        with (
            tc.tile_pool(name="pp", bufs=8, space="PSUM") as pp,
            tc.tile_pool(name="xt", bufs=2) as xtp,        # [128,16*cap] bf16
            tc.tile_pool(name="wu", bufs=3) as wup,        # quarters, 8KB
            tc.tile_pool(name="wd", bufs=2) as wdp,        # halves, 8KB
            tc.tile_pool(name="su", bufs=6) as sup,        # m-blocks, 4KB
            tc.tile_pool(name="sx", bufs=1) as sxp,        # xts / sd, 32KB
            tc.tile_pool(name="as_", bufs=1) as asp,       # a_s, 16KB
            tc.tile_pool(name="at", bufs=2) as atp,        # a-tiles, 8.6KB
            tc.tile_pool(name="os", bufs=2) as osp,        # out stage, 8KB
            tc.tile_pool(name="rl", bufs=4) as rlp,        # relu tmp, 1KB
            tc.tile_pool(name="cw", bufs=4) as cwp,
        ):
            # All inputs ride ONE queue (sync) so transfers arrive at full
            # engine bandwidth in exactly the order compute consumes them.
            # Outputs ride the scalar queue (same queue as the copies that
            # produce them).
            NQ = 4                       # wu quarters per expert
            QW = KH * DF // NQ           # 4096 cols per quarter
            NWH = 2                      # wd halves per expert
            HW_ = KD * H // NWH          # 8192 cols per half

            def load_wu(j):
                qs = []
                for q in range(NQ):
                    t = wup.tile([128, QW], BF16, tag="wu", name=f"wu{j}_{q}")
                    nc.sync.dma_start(
                        t[:], wu.ap()[j, :, q * QW:(q + 1) * QW])
                    qs.append(t)
                return qs

            def load_wd(j):
                hs = []
                for hh in range(NWH):
                    t = wdp.tile([128, HW_], BF16, tag="wd", name=f"wd{j}_{hh}")
                    nc.sync.dma_start(
                        t[:], wd.ap()[j, :, hh * HW_:(hh + 1) * HW_])
                    hs.append(t)
                return hs

            def load_xt(j):
                t = xtp.tile([128, KH * caps[j]], BF16, tag="xt",
                             name=f"xt{j}")
                nc.sync.dma_start(t[:], xt_r[j].ap()[:, :])
                return t

            # program-order state
            a_t = {}
            wd_t = {}

            def emit_up(j, wu_q):
                cap = caps[j]
                chunks, m_groups, order = _up_layout(cap)
                bidx = {km: i for i, km in enumerate(order)}
                at_tile = atp.tile([128, MD * CAPMAX], BF16, tag="at",
                                   name=f"at{j}")
                for mg in m_groups:
                    ps = {}
                    for m in mg:
                        for ci in range(len(chunks)):
                            ps[(m, ci)] = pp.tile([128, 512], F32, tag="pp",
                                                  name=f"pu{j}_{m}_{ci}")
                    for k in range(KH):
                        for m in mg:
                            bi = bidx[(k, m)]
                            q, r = divmod(bi * 128, QW)
                            wsl = wu_q[q][:, r:r + 128]
                            for ci, (off, w) in enumerate(chunks):
                                nc.tensor.matmul(
                                    ps[(m, ci)][:, :w], wsl,
                                    xt_t[j][:, k * cap + off:k * cap + off + w],
                                    start=(k == 0), stop=(k == KH - 1))
                    for m in mg:
                        for ci, (off, w) in enumerate(chunks):
                            r = rlp.tile([128, 512], BF16, tag="rl",
                                         name=f"r{j}_{m}_{ci}")
                            nc.scalar.activation(r[:, :w], ps[(m, ci)][:, :w],
                                                 relu)
                            nc.vector.tensor_mul(
                                at_tile[:, m * cap + off:m * cap + off + w],
                                r[:, :w], r[:, :w])
                a_t[j] = at_tile
                del xt_t[j]

            def emit_down(j):
                cap = caps[j]
                at_tile = a_t.pop(j)
                wdh = wd_t.pop(j)
                for tci in range(ntiles[j]):
                    t0 = tci * 128
                    M = min(128, cap - t0)
                    ps = [pp.tile([128, 512], F32, tag="pp",
                                  name=f"pd{j}_{tci}_{nn}") for nn in range(NH)]
                    for k2 in range(KD):
                        hh, r = divmod(k2 * H, HW_)
                        asl = at_tile[:, k2 * cap + t0:k2 * cap + t0 + M]
                        for nn in range(NH):
                            nc.tensor.matmul(
                                ps[nn][:M, :], asl,
                                wdh[hh][:, r + nn * 512:r + (nn + 1) * 512],
                                start=(k2 == 0), stop=(k2 == KD - 1))
                    # psum->sbuf copies (with combine-weight scale) split
                    # scalar/vector; each half DMAs from its own queue.
                    os_t = osp.tile([128, H], F32, tag="os",
                                    name=f"os{j}_{tci}")
                    csl = cw_t[j][:M, tci:tci + 1]
                    for nn in range(2):
                        nc.scalar.activation(
                            os_t[:M, nn * 512:(nn + 1) * 512], ps[nn][:M, :],
                            copyf, scale=csl)
                    nc.scalar.dma_start(
                        yr[j].ap()[:M, tci * H:tci * H + 1024], os_t[:M, :1024])
                    for nn in range(2, NH):
                        nc.vector.tensor_scalar_mul(
                            os_t[:M, nn * 512:(nn + 1) * 512], ps[nn][:M, :],
                            csl)
                    nc.gpsimd.dma_start(
                        yr[j].ap()[:M, tci * H + 1024:(tci + 1) * H],
                        os_t[:M, 1024:])

            # ================= schedule =================
            # Shared-up first: it needs the fewest input bytes (su m-block 0
            # + one xts quarter ≈ 1.5MB) so the PE starts ~4us in, and its
            # ~57us of compute covers the transfer of wu0/wd0/wu1.
            # DMA issue order on the sync queue == consumption order.
            su_t = [None] * MS
            su_t[0] = sup.tile([128, KH * 128], BF16, tag="su", name="su0")
            nc.sync.dma_start(su_t[0][:], su.ap()[:, 0:KH * 128])
            xts_t = sxp.tile([128, KH * T_LOC], BF16, tag="sx", name="xts")
            XQ = KH * T_LOC // 8
            for qq in range(8):
                nc.sync.dma_start(xts_t[:, qq * XQ:(qq + 1) * XQ],
                                  xts.ap()[:, qq * XQ:(qq + 1) * XQ])
            for m in range(1, MS):
                su_t[m] = sup.tile([128, KH * 128], BF16, tag="su",
                                   name=f"su{m}")
                nc.sync.dma_start(
                    su_t[m][:], su.ap()[:, m * (KH * 128):(m + 1) * (KH * 128)])
            xt_t = {0: load_xt(0)}
            wu_t = {0: load_wu(0)}
            wd_t[0] = load_wd(0)
            xt_t[1] = load_xt(1)
            wu_t[1] = load_wu(1)
            cw_t = []
            for j in range(SLOTS):
                t = cwp.tile([128, ntiles[j]], F32, tag="cw", name=f"cw{j}")
                nc.sync.dma_start(t[:], cw_r[j].ap()[:, :])
                cw_t.append(t)

            # ---------------- warm-up ----------------
            # ~50 dummy matmuls on never-written SBUF scratch keep the PE
            # busy through the DMA cold-start ramp, flipping the HAM clock
            # gate to 2.4GHz before real data lands (idle >3.4us would
            # re-throttle it).  Results go to a scratch PSUM, never read.
            warm_sb = rlp.tile([128, 512], BF16, tag="rl", name="warm_sb")
            nc.vector.memset(warm_sb[:], 0.0)
            warm_ps = pp.tile([128, 256], F32, tag="pp", name="warm_ps")
            for wi in range(48):
                nc.tensor.matmul(warm_ps[:], warm_sb[:, :128],
                                 warm_sb[:, 128:384], start=True, stop=True,
                                 skip_group_check=True)

            # ---------------- shared expert up ----------------
            # n-inner-of-m, k innermost: (m0, n0) needs only su block 0 and
            # the first xts quarter, so compute tracks the DMA wave.
            a_s = asp.tile([128, MS * T_LOC], BF16, tag="as", name="as")
            for m in range(MS):
                for n in range(NT):
                    ps = pp.tile([128, 512], F32, tag="pp", name=f"psu{m}_{n}")
                    for k in range(KH):
                        nc.tensor.matmul(
                            ps[:], su_t[m][:, k * 128:(k + 1) * 128],
                            xts_t[:, n * (KH * 512) + k * 512:
                                  n * (KH * 512) + (k + 1) * 512],
                            start=(k == 0), stop=(k == KH - 1))
                    r = rlp.tile([128, 512], BF16, tag="rl", name=f"rs{m}_{n}")
                    nc.scalar.activation(r[:], ps[:], relu)
                    nc.vector.tensor_mul(
                        a_s[:, m * T_LOC + n * 512:m * T_LOC + (n + 1) * 512],
                        r[:], r[:])

            # ---------------- routed experts ----------------
            # wd(j+1) is emitted before the xt/wu(j+2) prefetches: the
            # latter park the queue on ring-buffer reuse, and wd must not
            # queue behind that park.
            for j in range(SLOTS):
                emit_up(j, wu_t.pop(j))
                if j + 1 < SLOTS:
                    wd_t[j + 1] = load_wd(j + 1)
                if j + 2 < SLOTS:
                    xt_t[j + 2] = load_xt(j + 2)
                    wu_t[j + 2] = load_wu(j + 2)
                if j == 2:
                    # sd reuses the xts buffer (freed at shared-up end); late
                    # emission avoids parking the queue on that reuse.
                    sd_t = sxp.tile([128, KS * H], BF16, tag="sx", name="sd")
                    nc.sync.dma_start(sd_t[:], sd.ap()[:, :])
                emit_down(j)

            # ---------------- shared expert down ----------------
            for tci in range(T_LOC // 128):
                t0 = tci * 128
                ps = [pp.tile([128, 512], F32, tag="pp", name=f"psd{tci}_{nn}")
                      for nn in range(NH)]
                for k2 in range(KS):
                    asl = a_s[:, k2 * T_LOC + t0:k2 * T_LOC + t0 + 128]
                    for nn in range(NH):
                        nc.tensor.matmul(
                            ps[nn][:], asl,
                            sd_t[:, k2 * H + nn * 512:k2 * H + (nn + 1) * 512],
                            start=(k2 == 0), stop=(k2 == KS - 1))
                os_t = osp.tile([128, H], F32, tag="os", name=f"oss{tci}")
                for nn in range(2):
                    nc.scalar.activation(os_t[:, nn * 512:(nn + 1) * 512],
                                         ps[nn][:], copyf)
                nc.scalar.dma_start(ys.ap()[:, tci * H:tci * H + 1024],
                                    os_t[:, :1024])
                for nn in range(2, NH):
                    nc.vector.tensor_copy(os_t[:, nn * 512:(nn + 1) * 512],
                                          ps[nn][:])
                nc.gpsimd.dma_start(ys.ap()[:, tci * H + 1024:(tci + 1) * H],
                                    os_t[:, 1024:])

    nc.compile()
    return nc


def _pack_pm(mat, kt):
    """[kt*128, C] row-major -> [128, kt*C] partition-major (k-major cols)."""
    k128, c = mat.shape
    assert k128 == kt * 128
    return np.ascontiguousarray(
        mat.reshape(kt, 128, c).transpose(1, 0, 2).reshape(128, kt * c))


def kernel(x, router_w, router_b, w_up, w_down, shared_up, shared_down):
    global LAST_RESULTS, LAST_EXEC_NS
    x = np.asarray(x, dtype=np.float32)
    router_w = np.asarray(router_w, dtype=np.float32)
    router_b = np.asarray(router_b, dtype=np.float32)
    w_up = np.asarray(w_up, dtype=np.float32)
    w_down = np.asarray(w_down, dtype=np.float32)
    shared_up = np.asarray(shared_up, dtype=np.float32)
    shared_down = np.asarray(shared_down, dtype=np.float32)

    tidx, tw = _route_host(x, router_w, router_b)

    tok_of = [None] * E
    wgt_of = [None] * E
    for e in range(E):
        rows, cols = np.nonzero(tidx == e)
        tok_of[e] = rows
        wgt_of[e] = tw[rows, cols]
    counts = np.array([len(tok_of[e]) for e in range(E)])

    # bin-pack: rank groups of 8 per slot; greedy core assignment for balance
    order = np.argsort(-counts, kind="stable")
    assign = np.zeros((N_CORES, SLOTS), dtype=np.int64)
    core_load = np.zeros(N_CORES, dtype=np.int64)
    caps = []
    for j in range(SLOTS):
        grp = order[j * N_CORES:(j + 1) * N_CORES]
        caps.append(int(_roundup(max(int(counts[grp].max()), 16), 8)))
        cores_by_load = np.argsort(core_load, kind="stable")
        for i, e in enumerate(grp):  # grp is desc; pair big with least-loaded
            c = cores_by_load[i]
            assign[c, j] = e
            core_load[c] += counts[e]
    caps = tuple(caps)
    ntiles = [-(-c // 128) for c in caps]

    np_bf = mybir.dt.np(BF16)
    xt_full = np.ascontiguousarray(x.T).astype(np_bf)       # [H, T]
    su_cast = shared_up.astype(np_bf)
    sd_cast = shared_down.astype(np_bf)

    # shared-up packed m-major: [128, m*(KH*128) + k*128 + d]
    su_parts = []
    for r_tp in range(TP_S):
        blk = su_cast[:, r_tp * DS_LOC:(r_tp + 1) * DS_LOC]  # [H, DS_LOC]
        b3 = blk.reshape(KH, 128, MS, 128)  # [k, p, m, d]
        cols = b3.transpose(1, 2, 0, 3).reshape(128, MS * KH * 128)
        su_parts.append(np.ascontiguousarray(cols))
    sd_parts = [
        _pack_pm(sd_cast[r_tp * DS_LOC:(r_tp + 1) * DS_LOC, :], KS)
        for r_tp in range(TP_S)]
    # xts: n-chunk-major, [128, n*(KH*512) + k*512 + tt]
    xts_parts = []
    for g in range(DP_S):
        blocks = [
            _pack_pm(xt_full[:, g * T_LOC + n * 512:
                             g * T_LOC + (n + 1) * 512], KH)
            for n in range(NT)]
        xts_parts.append(np.ascontiguousarray(np.concatenate(blocks, axis=1)))

    in_maps = []
    for c in range(N_CORES):
        m = {}
        exp_ids = assign[c]
        wu_blocks = []
        wd_blocks = []
        for j in range(SLOTS):
            e = exp_ids[j]
            n = counts[e]
            cap = caps[j]
            # xt: [128, k*cap + c]
            xt_cj = np.zeros((H, cap), dtype=np_bf)
            xt_cj[:, :n] = xt_full[:, tok_of[e]]
            m[f"xt{j}"] = _pack_pm(xt_cj, KH)
            # cw: [128, ntiles]
            cw_cj = np.zeros((128 * ntiles[j],), dtype=np.float32)
            cw_cj[:n] = wgt_of[e]
            m[f"cw{j}"] = np.ascontiguousarray(
                cw_cj.reshape(ntiles[j], 128).T)
            # wu: consumption-order 128-col blocks (must match _up_layout)
            chunks_j, _, _ = _up_layout(cap)
            we = w_up[e].astype(np_bf)          # [H, DF]
            if len(chunks_j) == 1:
                # order (k, m): col = (k*8 + m)*128 + f
                wcols = we.reshape(KH, 128, MD, 128).transpose(
                    1, 0, 2, 3).reshape(128, KH * DF)
            else:
                # order (half, k, mi): col = (half*64 + k*4 + mi)*128 + f
                wcols = we.reshape(KH, 128, 2, MD // 2, 128).transpose(
                    1, 2, 0, 3, 4).reshape(128, KH * DF)
            wu_blocks.append(np.ascontiguousarray(wcols))
            # wd: [128, k2*H + h]
            wd_blocks.append(_pack_pm(w_down[e].astype(np_bf), KD))
        m["wu"] = np.ascontiguousarray(np.stack(wu_blocks))
        m["wd"] = np.ascontiguousarray(np.stack(wd_blocks))
        r_tp = c % TP_S
        g_dp = c // TP_S
        m["su"] = su_parts[r_tp]
        m["sd"] = sd_parts[r_tp]
        m["xts"] = xts_parts[g_dp]
        in_maps.append(m)

    key = (caps,)
    nc = _PROG_CACHE.get(key)
    if nc is None:
        nc = _build_program(caps)
        _PROG_CACHE[key] = nc

    res = run_bass_kernel_spmd(nc, in_maps, list(range(N_CORES)))
    LAST_RESULTS = res
    LAST_EXEC_NS = res.exec_time_ns

    out = np.zeros((T, H), dtype=np.float64)
    for c in range(N_CORES):
        g_dp = c // TP_S
        ys_c = res.results[c]["ys"].reshape(128, T_LOC // 128, H)
        ys_c = ys_c.transpose(1, 0, 2).reshape(T_LOC, H)
        out[g_dp * T_LOC:(g_dp + 1) * T_LOC] += ys_c.astype(np.float64)
        for j in range(SLOTS):
            e = assign[c, j]
            n = counts[e]
            if n:
                yr_c = res.results[c][f"yr{j}"].reshape(128, ntiles[j], H)
                yr_c = yr_c.transpose(1, 0, 2).reshape(ntiles[j] * 128, H)
                out[tok_of[e]] += yr_c[:n].astype(np.float64)
    return out.astype(np.float32)
